# revision 2
# baseline (speedup 1.0000x reference)
"""Distributed multi-head attention (B=2, L=2048, D=4096, H=32) on 8 TRN2 NeuronCores.

Strategy: tensor-parallel over heads (4 heads/core) for QKV+attention, then an
AllToAll that trades head-dims for token-slices so o_proj is token-sharded
(each core computes out[:, its 512 tokens] with the full Wo) — the AllToAll
moves 4 MB/core instead of the 64 MB/core an output AllReduce would.

All matmuls run in bf16 on the TensorEngine (f32 PSUM accumulation).
Host-side prep: transpose/permute/tile weights and x into DMA-friendly
partition-major layouts, pre-cast to bf16. Host post: concatenate the 8
token-shards and transpose. Verified rel-err vs f32 reference ~6e-3.

RoPE trick: Q/K output columns are permuted host-side (per head: even dims
then odd dims, pairs of heads interleaved into 128-row tiles) so the rotation
becomes full-width [128, t] vector ops with no partition-pair shuffles.
"""

import sys

if "/opt/trn_rl_repo" not in sys.path:
    sys.path.insert(0, "/opt/trn_rl_repo")

from contextlib import ExitStack

import ml_dtypes
import numpy as np

import concourse.bass as bass
import concourse.tile as tile
from concourse import bacc, mybir
from concourse import bass_utils

BF16 = mybir.dt.bfloat16
F32 = mybir.dt.float32
NPBF16 = ml_dtypes.bfloat16

NCORES = 8
B, L, D, H, HD = 2, 2048, 4096, 32, 128
T = B * L              # 4096 global tokens
NH = H // NCORES       # 4 heads per core
OC = NH * HD           # 512 projection dims per core
KT = D // 128          # 32 contraction tiles over D
LT = L // 128          # 16 key tiles per batch
TG = 512               # phase-1 token-group width
NG = T // TG           # 8 groups
SH = T // NCORES       # 512 output tokens per core
SHB = SH // B          # 256 per batch
SCALE = 1.0 / float(np.sqrt(HD))

EXP_F = mybir.ActivationFunctionType.Exp


def build_nc():
    nc = bacc.Bacc("TRN2", target_bir_lowering=False, debug=False,
                   num_devices=NCORES)

    # ---- I/O (per-core shards, host-pretiled, bf16) ----
    xT = nc.dram_tensor("xT", [KT, 128, T], BF16, kind="ExternalInput")
    wq = nc.dram_tensor("wq", [128, NH, KT, 128], BF16, kind="ExternalInput")
    wk = nc.dram_tensor("wk", [128, NH, KT, 128], BF16, kind="ExternalInput")
    wv = nc.dram_tensor("wv", [128, KT, OC], BF16, kind="ExternalInput")
    wo = nc.dram_tensor("wo", [D // 128, 128, KT, 128], BF16, kind="ExternalInput")
    cs = nc.dram_tensor("cs", [128, L], F32, kind="ExternalInput")
    sn = nc.dram_tensor("sn", [128, L], F32, kind="ExternalInput")
    ones = nc.dram_tensor("ones", [128, 128], BF16, kind="ExternalInput")
    out = nc.dram_tensor("out", [D, SH], F32, kind="ExternalOutput")

    # ---- internal DRAM (spills + collective bounce) ----
    qsp = [nc.dram_tensor(f"qsp{b}", [NH, 128, L], BF16) for b in range(B)]
    ksp = [nc.dram_tensor(f"ksp{b}", [NH, 128, L], BF16) for b in range(B)]
    vsp = [nc.dram_tensor(f"vsp{b}", [LT, 128, OC], BF16) for b in range(B)]
    a2a_in = [nc.dram_tensor(f"a2ai{b}", [NCORES, OC, SHB], BF16) for b in range(B)]
    a2a_out = [nc.dram_tensor(f"a2ao{b}", [NCORES, OC, SHB], BF16)
               for b in range(B)]

    with tile.TileContext(nc) as tc, ExitStack() as ctx:
        singles = ctx.enter_context(tc.tile_pool(name="singles", bufs=1))
        ones_sb = singles.tile([128, 128], BF16, name="ones")
        nc.sync.dma_start(ones_sb[:], ones[:, :])

        # ================= Phase 1: QKV projections + RoPE =================
        with ExitStack() as p1:
            wpool = p1.enter_context(tc.tile_pool(name="w", bufs=1))
            wq_sb = wpool.tile([128, NH, KT, 128], BF16, name="wq")
            nc.sync.dma_start(wq_sb[:], wq[:, :, :, :])
            wk_sb = wpool.tile([128, NH, KT, 128], BF16, name="wk")
            nc.sync.dma_start(wk_sb[:], wk[:, :, :, :])
            wv_sb = wpool.tile([128, KT, OC], BF16, name="wv")
            nc.sync.dma_start(wv_sb[:], wv[:, :, :])

            xpool = p1.enter_context(tc.tile_pool(name="xg", bufs=2))
            cpool = p1.enter_context(tc.tile_pool(name="csg", bufs=2))
            tmp = p1.enter_context(tc.tile_pool(name="tmp", bufs=8))
            st = p1.enter_context(tc.tile_pool(name="st", bufs=6))
            ps1 = p1.enter_context(tc.tile_pool(name="ps1", bufs=6, space="PSUM"))

            for g in range(NG):
                b = g // (NG // B)
                pos0 = (g % (NG // B)) * TG          # position within batch
                xg = xpool.tile([128, KT, TG], BF16, name="xg")
                for kt in range(KT):
                    nc.sync.dma_start(xg[:, kt, :], xT[kt, :, g * TG:(g + 1) * TG])
                csg = cpool.tile([128, TG], F32, name="csg")
                nc.sync.dma_start(csg[:], cs[:, pos0:pos0 + TG])
                sng = cpool.tile([128, TG], F32, name="sng")
                nc.sync.dma_start(sng[:], sn[:, pos0:pos0 + TG])

                # Q and K with fused RoPE
                for wsb, sp in ((wq_sb, qsp[b]), (wk_sb, ksp[b])):
                    for pr in range(NH // 2):
                        p_re = ps1.tile([128, TG], F32, name="ps1")
                        p_im = ps1.tile([128, TG], F32, name="ps1")
                        for kt in range(KT):
                            nc.tensor.matmul(p_re[:], wsb[:, 2 * pr, kt, :],
                                             xg[:, kt, :],
                                             start=(kt == 0), stop=(kt == KT - 1))
                        for kt in range(KT):
                            nc.tensor.matmul(p_im[:], wsb[:, 2 * pr + 1, kt, :],
                                             xg[:, kt, :],
                                             start=(kt == 0), stop=(kt == KT - 1))
                        t1 = tmp.tile([128, TG], F32, name="tmp")
                        t2 = tmp.tile([128, TG], F32, name="tmp")
                        t3 = tmp.tile([128, TG], F32, name="tmp")
                        t4 = tmp.tile([128, TG], F32, name="tmp")
                        o_re = st.tile([128, TG], BF16, name="st")
                        o_im = st.tile([128, TG], BF16, name="st")
                        nc.vector.tensor_mul(t1[:], p_re[:], csg[:])
                        nc.vector.tensor_mul(t2[:], p_im[:], sng[:])
                        nc.vector.tensor_sub(o_re[:], t1[:], t2[:])
                        nc.vector.tensor_mul(t3[:], p_re[:], sng[:])
                        nc.vector.tensor_mul(t4[:], p_im[:], csg[:])
                        nc.vector.tensor_add(o_im[:], t3[:], t4[:])
                        ha, hb = 2 * pr, 2 * pr + 1
                        nc.sync.dma_start(sp[ha, 0:64, pos0:pos0 + TG], o_re[0:64, :])
                        nc.sync.dma_start(sp[hb, 0:64, pos0:pos0 + TG], o_re[64:128, :])
                        nc.sync.dma_start(sp[ha, 64:128, pos0:pos0 + TG], o_im[0:64, :])
                        nc.sync.dma_start(sp[hb, 64:128, pos0:pos0 + TG], o_im[64:128, :])

                # V (layout [t, oc])
                for sub in range(TG // 128):
                    pv = ps1.tile([128, OC], F32, name="ps1")
                    for kt in range(KT):
                        nc.tensor.matmul(pv[:], xg[:, kt, sub * 128:(sub + 1) * 128],
                                         wv_sb[:, kt, :],
                                         start=(kt == 0), stop=(kt == KT - 1))
                    vo = st.tile([128, OC], BF16, name="st")
                    nc.scalar.copy(vo[:], pv[:])
                    tt = pos0 // 128 + sub
                    nc.sync.dma_start(vsp[b][tt, :, :], vo[:])

        # ================= Phase 2: attention (scores/softmax/PV) ==========
        with ExitStack() as p2:
            qk = p2.enter_context(tc.tile_pool(name="qk", bufs=2))
            vbp = p2.enter_context(tc.tile_pool(name="vb", bufs=2))
            ep = p2.enter_context(tc.tile_pool(name="ep", bufs=3))
            rc = p2.enter_context(tc.tile_pool(name="rc", bufs=4))
            ao = p2.enter_context(tc.tile_pool(name="ao", bufs=3))
            ps_s = p2.enter_context(tc.tile_pool(name="ps_s", bufs=2, space="PSUM"))
            ps_pv = p2.enter_context(tc.tile_pool(name="ps_pv", bufs=2, space="PSUM"))
            ps_r = p2.enter_context(tc.tile_pool(name="ps_r", bufs=2, space="PSUM"))

            for b in range(B):
                vb = vbp.tile([128, LT, OC], BF16, name="vb")
                for tt in range(LT):
                    nc.sync.dma_start(vb[:, tt, :], vsp[b][tt, :, :])
                for h in range(NH):
                    q_sb = qk.tile([128, L], BF16, name="q")
                    nc.sync.dma_start(q_sb[:], qsp[b][h, :, :])
                    k_sb = qk.tile([128, L], BF16, name="k")
                    nc.sync.dma_start(k_sb[:], ksp[b][h, :, :])
                    for half in range(2):
                        q0 = half * 1024
                        pvs = [ps_pv.tile([128, 512], F32, name="ps_pv")
                               for _ in range(2)]
                        rs = [ps_r.tile([128, 512], F32, name="ps_r")
                              for _ in range(2)]
                        for kt in range(LT):
                            s_ps = ps_s.tile([128, 1024], F32, name="ps_s")
                            nc.tensor.matmul(s_ps[:, 0:512],
                                             k_sb[:, kt * 128:(kt + 1) * 128],
                                             q_sb[:, q0:q0 + 512],
                                             start=True, stop=True)
                            nc.tensor.matmul(s_ps[:, 512:1024],
                                             k_sb[:, kt * 128:(kt + 1) * 128],
                                             q_sb[:, q0 + 512:q0 + 1024],
                                             start=True, stop=True)
                            e_t = ep.tile([128, 1024], BF16, name="ep")
                            nc.scalar.activation(e_t[:], s_ps[:], EXP_F, scale=SCALE)
                            first, last = (kt == 0), (kt == LT - 1)
                            for c in range(2):
                                nc.tensor.matmul(pvs[c][:],
                                                 vb[:, kt, h * 128:(h + 1) * 128],
                                                 e_t[:, c * 512:(c + 1) * 512],
                                                 start=first, stop=last)
                                nc.tensor.matmul(rs[c][:], ones_sb[:],
                                                 e_t[:, c * 512:(c + 1) * 512],
                                                 start=first, stop=last)
                        for c in range(2):
                            rec = rc.tile([128, 512], F32, name="rc")
                            nc.vector.reciprocal_approx_fast(out=rec[:], in_=rs[c][:])
                            at = ao.tile([128, 512], BF16, name="ao")
                            nc.vector.tensor_mul(at[:], pvs[c][:], rec[:])
                            ci = half * 2 + c
                            nc.sync.dma_start(
                                a2a_in[b][2 * ci, h * 128:(h + 1) * 128, :],
                                at[:, 0:SHB])
                            nc.sync.dma_start(
                                a2a_in[b][2 * ci + 1, h * 128:(h + 1) * 128, :],
                                at[:, SHB:2 * SHB])
                nc.gpsimd.collective_compute(
                    "AllToAll", mybir.AluOpType.bypass,
                    replica_groups=[list(range(NCORES))],
                    ins=[a2a_in[b].ap().opt()],
                    outs=[a2a_out[b].ap().opt()],
                )

        # ================= Phase 3: o_proj on this core's token shard ======
        with ExitStack() as p3:
            rhp = p3.enter_context(tc.tile_pool(name="rh", bufs=2))
            wop = p3.enter_context(tc.tile_pool(name="wo", bufs=3))
            osb = p3.enter_context(tc.tile_pool(name="osb", bufs=4))
            ps_o = p3.enter_context(tc.tile_pool(name="ps_o", bufs=2, space="PSUM"))
            for b in range(B):
                rh = rhp.tile([128, KT, SHB], BF16, name="rh")
                a2o = a2a_out[b].ap().rearrange("j d t -> (j d) t")
                for kt in range(KT):
                    nc.sync.dma_start(rh[:, kt, :], a2o[kt * 128:(kt + 1) * 128, :])
                for ot in range(D // 128):
                    wot = wop.tile([128, KT, 128], BF16, name="wo")
                    nc.sync.dma_start(wot[:], wo[ot, :, :, :])
                    po = ps_o.tile([128, SHB], F32, name="ps_o")
                    for kt in range(KT):
                        nc.tensor.matmul(po[:], wot[:, kt, :], rh[:, kt, :],
                                         start=(kt == 0), stop=(kt == KT - 1))
                    o_sb = osb.tile([128, SHB], F32, name="osb")
                    nc.scalar.copy(o_sb[:], po[:])
                    nc.sync.dma_start(
                        out[ot * 128:(ot + 1) * 128, b * SHB:(b + 1) * SHB],
                        o_sb[:])

    nc.compile()
    return nc


def _qk_row_perm():
    # local row order: [h0re|h1re],[h0im|h1im],[h2re|h3re],[h2im|h3im]
    rows = []
    for pr in range(NH // 2):
        ha, hb = 2 * pr, 2 * pr + 1
        rows += [ha * HD + 2 * i for i in range(HD // 2)]
        rows += [hb * HD + 2 * i for i in range(HD // 2)]
        rows += [ha * HD + 2 * i + 1 for i in range(HD // 2)]
        rows += [hb * HD + 2 * i + 1 for i in range(HD // 2)]
    return np.array(rows)


def _prep_inputs(x, freqs_cos, freqs_sin, Wq, Wk, Wv, Wo):
    x = np.asarray(x, np.float32).reshape(T, D)
    Wq, Wk, Wv, Wo = (np.asarray(w, np.float32) for w in (Wq, Wk, Wv, Wo))
    fc = np.asarray(freqs_cos, np.float32)
    fs = np.asarray(freqs_sin, np.float32)

    # shared tensors
    xT = np.ascontiguousarray(
        x.reshape(T, KT, 128).transpose(1, 2, 0)).astype(NPBF16)        # [KT,128,T]
    woh = np.ascontiguousarray(
        Wo.reshape(D // 128, 128, KT, 128).transpose(0, 3, 2, 1)).astype(NPBF16)
    csh = np.ascontiguousarray(np.concatenate([fc.T, fc.T], 0))          # [128,L]
    snh = np.ascontiguousarray(np.concatenate([fs.T, fs.T], 0))
    ones = np.ones([128, 128], NPBF16)

    perm = _qk_row_perm()
    in_maps = []
    for i in range(NCORES):
        rows = slice(OC * i, OC * (i + 1))
        wqi = Wq[rows][perm]                                             # [512, D]
        wki = Wk[rows][perm]
        wqh = np.ascontiguousarray(
            wqi.reshape(NH, 128, KT, 128).transpose(3, 0, 2, 1)).astype(NPBF16)
        wkh = np.ascontiguousarray(
            wki.reshape(NH, 128, KT, 128).transpose(3, 0, 2, 1)).astype(NPBF16)
        wvh = np.ascontiguousarray(
            Wv[rows].reshape(OC, KT, 128).transpose(2, 1, 0)).astype(NPBF16)
        in_maps.append({
            "xT": xT, "wq": wqh, "wk": wkh, "wv": wvh, "wo": woh,
            "cs": csh, "sn": snh, "ones": ones,
        })
    return in_maps


_NC_CACHE = None


def _get_nc():
    global _NC_CACHE
    if _NC_CACHE is None:
        _NC_CACHE = build_nc()
    return _NC_CACHE


def _run(in_maps, trace=False):
    nc = _get_nc()
    res = bass_utils.run_bass_kernel_spmd(
        nc, in_maps, core_ids=list(range(NCORES)), trace=trace)
    return res


def _assemble(results):
    out = np.empty((B, L, D), np.float32)
    for i in range(NCORES):
        o = results[i]["out"]                       # [D, SH] f32
        for b in range(B):
            out[b, SHB * i:SHB * (i + 1), :] = o[:, b * SHB:(b + 1) * SHB].T
    return out


def kernel(x, freqs_cos, freqs_sin, Wq, Wk, Wv, Wo):
    in_maps = _prep_inputs(x, freqs_cos, freqs_sin, Wq, Wk, Wv, Wo)
    res = _run(in_maps, trace=False)
    return _assemble(res.results)


# revision 5
# speedup vs baseline: 1.0058x; 1.0058x over previous
"""Distributed multi-head attention (B=2, L=2048, D=4096, H=32) on 8 TRN2 NeuronCores.

Strategy: tensor-parallel over heads (4 heads/core) for QKV+attention, then an
AllToAll that trades head-dims for token-slices so o_proj is token-sharded
(each core computes out[:, its 512 tokens] with the full Wo) — the AllToAll
moves 4 MB/core instead of the 64 MB/core an output AllReduce would.

All matmuls run in bf16 on the TensorEngine (f32 PSUM accumulation).
Host-side prep: transpose/permute/tile weights and x into DMA-friendly
partition-major layouts, pre-cast to bf16. Host post: concatenate the 8
token-shards and transpose. Verified rel-err vs f32 reference ~6e-3.

RoPE trick: Q/K output columns are permuted host-side (per head: even dims
then odd dims, pairs of heads interleaved into 128-row tiles) so the rotation
becomes full-width [128, t] vector ops with no partition-pair shuffles.

Perf notes (iter2): all multi-tile loads are single dma_starts (one semaphore
wait instead of one per matmul — per-MM issue measured 266ns vs the 225ns
floor when every kt-tile MM waits its own DMA sem); o_proj is one N=512 pass
over both batches; the AllToAll is split 4 ways (per batch x head-pair) so
only the last ~1MB exchange is exposed; stores go on the GpSimd DMA queue,
loads on Sync.
"""

import sys

if "/opt/trn_rl_repo" not in sys.path:
    sys.path.insert(0, "/opt/trn_rl_repo")

from contextlib import ExitStack

import ml_dtypes
import numpy as np

import concourse.bass as bass
import concourse.tile as tile
from concourse import bacc, mybir
from concourse import bass_utils

BF16 = mybir.dt.bfloat16
F32 = mybir.dt.float32
NPBF16 = ml_dtypes.bfloat16

NCORES = 8
B, L, D, H, HD = 2, 2048, 4096, 32, 128
T = B * L              # 4096 global tokens
NH = H // NCORES       # 4 heads per core
OC = NH * HD           # 512 projection dims per core
KT = D // 128          # 32 contraction tiles over D
LT = L // 128          # 16 key tiles per batch
TG = 512               # phase-1 token-group width
NG = T // TG           # 8 groups
SH = T // NCORES       # 512 output tokens per core
SHB = SH // B          # 256 per batch
SCALE = 1.0 / float(np.sqrt(HD))

EXP_F = mybir.ActivationFunctionType.Exp


def build_nc():
    nc = bacc.Bacc("TRN2", target_bir_lowering=False, debug=False,
                   num_devices=NCORES)

    # ---- I/O (per-core shards, host-pretiled, bf16) ----
    xT = nc.dram_tensor("xT", [KT, 128, T], BF16, kind="ExternalInput")
    wq = nc.dram_tensor("wq", [128, NH, KT, 128], BF16, kind="ExternalInput")
    wk = nc.dram_tensor("wk", [128, NH, KT, 128], BF16, kind="ExternalInput")
    wv = nc.dram_tensor("wv", [128, KT, OC], BF16, kind="ExternalInput")
    wo = nc.dram_tensor("wo", [D // 128, 128, KT, 128], BF16, kind="ExternalInput")
    cs = nc.dram_tensor("cs", [128, L], F32, kind="ExternalInput")
    sn = nc.dram_tensor("sn", [128, L], F32, kind="ExternalInput")
    ones = nc.dram_tensor("ones", [128, 128], BF16, kind="ExternalInput")
    out = nc.dram_tensor("out", [D, SH], F32, kind="ExternalOutput")

    # ---- internal DRAM (spills + collective bounce) ----
    qsp = [nc.dram_tensor(f"qsp{b}", [NH, 128, L], BF16) for b in range(B)]
    ksp = [nc.dram_tensor(f"ksp{b}", [NH, 128, L], BF16) for b in range(B)]
    vsp = [nc.dram_tensor(f"vsp{b}", [LT, 128, OC], BF16) for b in range(B)]
    # AllToAll split by (batch, head-pair): [shard, 2*HD dims, SHB tokens]
    a2a_in = [[nc.dram_tensor(f"a2ai{b}_{hp}", [NCORES, 2 * HD, SHB], BF16)
               for hp in range(NH // 2)] for b in range(B)]
    a2a_out = [[nc.dram_tensor(f"a2ao{b}_{hp}", [NCORES, 2 * HD, SHB], BF16)
                for hp in range(NH // 2)] for b in range(B)]

    with tile.TileContext(nc) as tc, ExitStack() as ctx:
        singles = ctx.enter_context(tc.tile_pool(name="singles", bufs=1))
        ones_sb = singles.tile([128, 128], BF16, name="ones")
        nc.sync.dma_start(ones_sb[:], ones[:, :])

        # ================= Phase 1: QKV projections + RoPE =================
        with ExitStack() as p1:
            wpool = p1.enter_context(tc.tile_pool(name="w", bufs=1))
            wq_sb = wpool.tile([128, NH, KT, 128], BF16, name="wq")
            nc.sync.dma_start(wq_sb[:], wq[:, :, :, :])
            wk_sb = wpool.tile([128, NH, KT, 128], BF16, name="wk")
            nc.sync.dma_start(wk_sb[:], wk[:, :, :, :])
            wv_sb = wpool.tile([128, KT, OC], BF16, name="wv")
            nc.sync.dma_start(wv_sb[:], wv[:, :, :])

            xpool = p1.enter_context(tc.tile_pool(name="xg", bufs=2))
            cpool = p1.enter_context(tc.tile_pool(name="csg", bufs=2))
            tmp = p1.enter_context(tc.tile_pool(name="tmp", bufs=8))
            st = p1.enter_context(tc.tile_pool(name="st", bufs=6))
            ps1 = p1.enter_context(tc.tile_pool(name="ps1", bufs=6, space="PSUM"))

            for g in range(NG):
                b = g // (NG // B)
                pos0 = (g % (NG // B)) * TG          # position within batch
                xg = xpool.tile([128, KT, TG], BF16, name="xg")
                nc.sync.dma_start(
                    xg[:, :, :],
                    xT[:, :, g * TG:(g + 1) * TG].transpose([1, 0, 2]))
                csg = cpool.tile([128, TG], F32, name="csg")
                nc.sync.dma_start(csg[:], cs[:, pos0:pos0 + TG])
                sng = cpool.tile([128, TG], F32, name="sng")
                nc.sync.dma_start(sng[:], sn[:, pos0:pos0 + TG])

                # Q and K with fused RoPE
                for wsb, sp in ((wq_sb, qsp[b]), (wk_sb, ksp[b])):
                    for pr in range(NH // 2):
                        p_re = ps1.tile([128, TG], F32, name="ps1")
                        p_im = ps1.tile([128, TG], F32, name="ps1")
                        for kt in range(KT):
                            nc.tensor.matmul(p_re[:], wsb[:, 2 * pr, kt, :],
                                             xg[:, kt, :],
                                             start=(kt == 0), stop=(kt == KT - 1))
                        for kt in range(KT):
                            nc.tensor.matmul(p_im[:], wsb[:, 2 * pr + 1, kt, :],
                                             xg[:, kt, :],
                                             start=(kt == 0), stop=(kt == KT - 1))
                        t1 = tmp.tile([128, TG], F32, name="tmp")
                        t2 = tmp.tile([128, TG], F32, name="tmp")
                        t3 = tmp.tile([128, TG], F32, name="tmp")
                        t4 = tmp.tile([128, TG], F32, name="tmp")
                        o_re = st.tile([128, TG], BF16, name="st")
                        o_im = st.tile([128, TG], BF16, name="st")
                        nc.vector.tensor_mul(t1[:], p_re[:], csg[:])
                        nc.vector.tensor_mul(t2[:], p_im[:], sng[:])
                        nc.vector.tensor_sub(o_re[:], t1[:], t2[:])
                        nc.vector.tensor_mul(t3[:], p_re[:], sng[:])
                        nc.vector.tensor_mul(t4[:], p_im[:], csg[:])
                        nc.vector.tensor_add(o_im[:], t3[:], t4[:])
                        ha, hb = 2 * pr, 2 * pr + 1
                        nc.gpsimd.dma_start(sp[ha, 0:64, pos0:pos0 + TG],
                                            o_re[0:64, :])
                        nc.gpsimd.dma_start(sp[hb, 0:64, pos0:pos0 + TG],
                                            o_re[64:128, :])
                        nc.gpsimd.dma_start(sp[ha, 64:128, pos0:pos0 + TG],
                                            o_im[0:64, :])
                        nc.gpsimd.dma_start(sp[hb, 64:128, pos0:pos0 + TG],
                                            o_im[64:128, :])

                # V (layout [t, oc])
                for sub in range(TG // 128):
                    pv = ps1.tile([128, OC], F32, name="ps1")
                    for kt in range(KT):
                        nc.tensor.matmul(pv[:], xg[:, kt, sub * 128:(sub + 1) * 128],
                                         wv_sb[:, kt, :],
                                         start=(kt == 0), stop=(kt == KT - 1))
                    vo = st.tile([128, OC], BF16, name="st")
                    nc.scalar.copy(vo[:], pv[:])
                    tt = pos0 // 128 + sub
                    nc.gpsimd.dma_start(vsp[b][tt, :, :], vo[:])

        # ================= Phase 2: attention (scores/softmax/PV) ==========
        with ExitStack() as p2:
            qk = p2.enter_context(tc.tile_pool(name="qk", bufs=2))
            vbp = p2.enter_context(tc.tile_pool(name="vb", bufs=2))
            ep = p2.enter_context(tc.tile_pool(name="ep", bufs=3))
            rc = p2.enter_context(tc.tile_pool(name="rc", bufs=4))
            ao = p2.enter_context(tc.tile_pool(name="ao", bufs=3))
            ps_s = p2.enter_context(tc.tile_pool(name="ps_s", bufs=2, space="PSUM"))
            ps_pv = p2.enter_context(tc.tile_pool(name="ps_pv", bufs=2, space="PSUM"))
            ps_r = p2.enter_context(tc.tile_pool(name="ps_r", bufs=2, space="PSUM"))

            for b in range(B):
                vb = vbp.tile([128, LT, OC], BF16, name="vb")
                nc.sync.dma_start(vb[:, :, :],
                                  vsp[b].ap().transpose([1, 0, 2]))
                for h in range(NH):
                    q_sb = qk.tile([128, L], BF16, name="q")
                    nc.sync.dma_start(q_sb[:], qsp[b][h, :, :])
                    k_sb = qk.tile([128, L], BF16, name="k")
                    nc.sync.dma_start(k_sb[:], ksp[b][h, :, :])
                    for half in range(2):
                        q0 = half * 1024
                        pvs = [ps_pv.tile([128, 512], F32, name="ps_pv")
                               for _ in range(2)]
                        rs = [ps_r.tile([128, 512], F32, name="ps_r")
                              for _ in range(2)]
                        for kt in range(LT):
                            s_ps = ps_s.tile([128, 1024], F32, name="ps_s")
                            nc.tensor.matmul(s_ps[:, 0:512],
                                             k_sb[:, kt * 128:(kt + 1) * 128],
                                             q_sb[:, q0:q0 + 512],
                                             start=True, stop=True)
                            nc.tensor.matmul(s_ps[:, 512:1024],
                                             k_sb[:, kt * 128:(kt + 1) * 128],
                                             q_sb[:, q0 + 512:q0 + 1024],
                                             start=True, stop=True)
                            e_t = ep.tile([128, 1024], BF16, name="ep")
                            nc.scalar.activation(e_t[:], s_ps[:], EXP_F, scale=SCALE)
                            first, last = (kt == 0), (kt == LT - 1)
                            for c in range(2):
                                nc.tensor.matmul(pvs[c][:],
                                                 vb[:, kt, h * 128:(h + 1) * 128],
                                                 e_t[:, c * 512:(c + 1) * 512],
                                                 start=first, stop=last)
                                nc.tensor.matmul(rs[c][:], ones_sb[:],
                                                 e_t[:, c * 512:(c + 1) * 512],
                                                 start=first, stop=last)
                        for c in range(2):
                            rec = rc.tile([128, 512], F32, name="rc")
                            nc.vector.reciprocal_approx_fast(out=rec[:], in_=rs[c][:])
                            at = ao.tile([128, 512], BF16, name="ao")
                            nc.vector.tensor_mul(at[:], pvs[c][:], rec[:])
                            ci = half * 2 + c
                            hp, hh = h // 2, h % 2
                            nc.gpsimd.dma_start(
                                a2a_in[b][hp][2 * ci, hh * 128:(hh + 1) * 128, :],
                                at[:, 0:SHB])
                            nc.gpsimd.dma_start(
                                a2a_in[b][hp][2 * ci + 1, hh * 128:(hh + 1) * 128, :],
                                at[:, SHB:2 * SHB])
                    if h % 2 == 1:
                        hp = h // 2
                        nc.gpsimd.collective_compute(
                            "AllToAll", mybir.AluOpType.bypass,
                            replica_groups=[list(range(NCORES))],
                            ins=[a2a_in[b][hp].ap().opt()],
                            outs=[a2a_out[b][hp].ap().opt()],
                        )

        # ================= Phase 3: o_proj on this core's token shard ======
        with ExitStack() as p3:
            rhp = p3.enter_context(tc.tile_pool(name="rh", bufs=1))
            wop = p3.enter_context(tc.tile_pool(name="wo", bufs=3))
            osb = p3.enter_context(tc.tile_pool(name="osb", bufs=4))
            ps_o = p3.enter_context(tc.tile_pool(name="ps_o", bufs=2, space="PSUM"))
            rh = rhp.tile([128, KT, SH], BF16, name="rh")
            rh4 = rh[:].rearrange("p (j f) t -> p j f t", f=4)   # [128,8,4,SH]
            for b in range(B):
                for hp in range(NH // 2):
                    # global kt = 4*j + 2*hp + kk
                    src = a2a_out[b][hp].ap().rearrange(
                        "j (kk p) t -> p j kk t", p=128)         # [128,8,2,SHB]
                    for kk in range(2):
                        nc.sync.dma_start(
                            rh4[:, :, 2 * hp + kk, b * SHB:(b + 1) * SHB],
                            src[:, :, kk, :])
            for ot in range(D // 128):
                wot = wop.tile([128, KT, 128], BF16, name="wo")
                nc.sync.dma_start(wot[:], wo[ot, :, :, :])
                po = ps_o.tile([128, SH], F32, name="ps_o")
                for kt in range(KT):
                    nc.tensor.matmul(po[:], wot[:, kt, :], rh[:, kt, :],
                                     start=(kt == 0), stop=(kt == KT - 1))
                o_sb = osb.tile([128, SH], F32, name="osb")
                nc.scalar.copy(o_sb[:], po[:])
                nc.gpsimd.dma_start(out[ot * 128:(ot + 1) * 128, :], o_sb[:])

    nc.compile()
    return nc


def _qk_row_perm():
    # local row order: [h0re|h1re],[h0im|h1im],[h2re|h3re],[h2im|h3im]
    rows = []
    for pr in range(NH // 2):
        ha, hb = 2 * pr, 2 * pr + 1
        rows += [ha * HD + 2 * i for i in range(HD // 2)]
        rows += [hb * HD + 2 * i for i in range(HD // 2)]
        rows += [ha * HD + 2 * i + 1 for i in range(HD // 2)]
        rows += [hb * HD + 2 * i + 1 for i in range(HD // 2)]
    return np.array(rows)


def _prep_inputs(x, freqs_cos, freqs_sin, Wq, Wk, Wv, Wo):
    x = np.asarray(x, np.float32).reshape(T, D)
    Wq, Wk, Wv, Wo = (np.asarray(w, np.float32) for w in (Wq, Wk, Wv, Wo))
    fc = np.asarray(freqs_cos, np.float32)
    fs = np.asarray(freqs_sin, np.float32)

    # shared tensors
    xT = np.ascontiguousarray(
        x.reshape(T, KT, 128).transpose(1, 2, 0)).astype(NPBF16)        # [KT,128,T]
    woh = np.ascontiguousarray(
        Wo.reshape(D // 128, 128, KT, 128).transpose(0, 3, 2, 1)).astype(NPBF16)
    csh = np.ascontiguousarray(np.concatenate([fc.T, fc.T], 0))          # [128,L]
    snh = np.ascontiguousarray(np.concatenate([fs.T, fs.T], 0))
    ones = np.ones([128, 128], NPBF16)

    perm = _qk_row_perm()
    in_maps = []
    for i in range(NCORES):
        rows = slice(OC * i, OC * (i + 1))
        wqi = Wq[rows][perm]                                             # [512, D]
        wki = Wk[rows][perm]
        wqh = np.ascontiguousarray(
            wqi.reshape(NH, 128, KT, 128).transpose(3, 0, 2, 1)).astype(NPBF16)
        wkh = np.ascontiguousarray(
            wki.reshape(NH, 128, KT, 128).transpose(3, 0, 2, 1)).astype(NPBF16)
        wvh = np.ascontiguousarray(
            Wv[rows].reshape(OC, KT, 128).transpose(2, 1, 0)).astype(NPBF16)
        in_maps.append({
            "xT": xT, "wq": wqh, "wk": wkh, "wv": wvh, "wo": woh,
            "cs": csh, "sn": snh, "ones": ones,
        })
    return in_maps


_NC_CACHE = None


def _get_nc():
    global _NC_CACHE
    if _NC_CACHE is None:
        _NC_CACHE = build_nc()
    return _NC_CACHE


def _run(in_maps, trace=False):
    nc = _get_nc()
    res = bass_utils.run_bass_kernel_spmd(
        nc, in_maps, core_ids=list(range(NCORES)), trace=trace)
    return res


def _assemble(results):
    out = np.empty((B, L, D), np.float32)
    for i in range(NCORES):
        o = results[i]["out"]                       # [D, SH] f32
        for b in range(B):
            out[b, SHB * i:SHB * (i + 1), :] = o[:, b * SHB:(b + 1) * SHB].T
    return out


def kernel(x, freqs_cos, freqs_sin, Wq, Wk, Wv, Wo):
    in_maps = _prep_inputs(x, freqs_cos, freqs_sin, Wq, Wk, Wv, Wo)
    res = _run(in_maps, trace=False)
    return _assemble(res.results)


# revision 8
# speedup vs baseline: 1.0733x; 1.0671x over previous
"""Distributed multi-head attention (B=2, L=2048, D=4096, H=32) on 8 TRN2 NeuronCores.

Strategy: tensor-parallel over heads (4 heads/core) for QKV+attention, then an
AllToAll that trades head-dims for token-slices so o_proj is token-sharded
(each core computes out[:, its 512 tokens] with the full Wo) — the AllToAll
moves 4 MB/core instead of the 64 MB/core an output AllReduce would.

All matmuls run in bf16 on the TensorEngine (f32 PSUM accumulation).
Host-side prep: transpose/permute/tile weights and x into DMA-friendly
partition-major layouts, pre-cast to bf16. Host post: concatenate the 8
token-shards and transpose. Verified rel-err vs f32 reference ~6e-3.

RoPE trick: Q/K output columns are permuted host-side (per head: even dims
then odd dims, pairs of heads interleaved into 128-row tiles) so the rotation
becomes full-width [128, t] vector ops with no partition-pair shuffles.

Perf notes (iter2): all multi-tile loads are single dma_starts (one semaphore
wait instead of one per matmul — per-MM issue measured 266ns vs the 225ns
floor when every kt-tile MM waits its own DMA sem); o_proj is one N=512 pass
over both batches; the AllToAll is split 4 ways (per batch x head-pair) so
only the last ~1MB exchange is exposed; stores go on the GpSimd DMA queue,
loads on Sync.
"""

import sys

if "/opt/trn_rl_repo" not in sys.path:
    sys.path.insert(0, "/opt/trn_rl_repo")

from contextlib import ExitStack

import ml_dtypes
import numpy as np

import concourse.bass as bass
import concourse.tile as tile
from concourse import bacc, mybir
from concourse import bass_utils

BF16 = mybir.dt.bfloat16
F32 = mybir.dt.float32
NPBF16 = ml_dtypes.bfloat16

NCORES = 8
B, L, D, H, HD = 2, 2048, 4096, 32, 128
T = B * L              # 4096 global tokens
NH = H // NCORES       # 4 heads per core
OC = NH * HD           # 512 projection dims per core
KT = D // 128          # 32 contraction tiles over D
LT = L // 128          # 16 key tiles per batch
TG = 512               # phase-1 token-group width
NG = T // TG           # 8 groups
SH = T // NCORES       # 512 output tokens per core
SHB = SH // B          # 256 per batch
SCALE = 1.0 / float(np.sqrt(HD))

EXP_F = mybir.ActivationFunctionType.Exp


def build_nc():
    nc = bacc.Bacc("TRN2", target_bir_lowering=False, debug=False,
                   num_devices=NCORES)

    # ---- I/O (per-core shards, host-pretiled, bf16) ----
    xT = nc.dram_tensor("xT", [KT, 128, T], BF16, kind="ExternalInput")
    wq = nc.dram_tensor("wq", [128, NH, KT, 128], BF16, kind="ExternalInput")
    wk = nc.dram_tensor("wk", [128, NH, KT, 128], BF16, kind="ExternalInput")
    wv = nc.dram_tensor("wv", [128, KT, OC], BF16, kind="ExternalInput")
    wo = nc.dram_tensor("wo", [D // 128, 128, KT, 128], BF16, kind="ExternalInput")
    cs = nc.dram_tensor("cs", [128, L], F32, kind="ExternalInput")
    sn = nc.dram_tensor("sn", [128, L], F32, kind="ExternalInput")
    ones = nc.dram_tensor("ones", [128, 128], BF16, kind="ExternalInput")
    out = nc.dram_tensor("out", [D, SH], F32, kind="ExternalOutput")

    # ---- internal DRAM (spills + collective bounce) ----
    qsp = [nc.dram_tensor(f"qsp{b}", [NH, 128, L], BF16) for b in range(B)]
    ksp = [nc.dram_tensor(f"ksp{b}", [NH, 128, L], BF16) for b in range(B)]
    vsp = [nc.dram_tensor(f"vsp{b}", [LT, 128, OC], BF16) for b in range(B)]
    # AllToAll split by (batch, head): [shard, HD dims, SHB tokens]
    a2a_in = [[nc.dram_tensor(f"a2ai{b}_{h}", [NCORES, HD, SHB], BF16)
               for h in range(NH)] for b in range(B)]
    a2a_out = [[nc.dram_tensor(f"a2ao{b}_{h}", [NCORES, HD, SHB], BF16)
                for h in range(NH)] for b in range(B)]

    with tile.TileContext(nc) as tc, ExitStack() as ctx:
        singles = ctx.enter_context(tc.tile_pool(name="singles", bufs=1))
        ones_sb = singles.tile([128, 128], BF16, name="ones")
        nc.sync.dma_start(ones_sb[:], ones[:, :])

        # ================= Phase 1: QKV projections + RoPE =================
        with ExitStack() as p1:
            wpool = p1.enter_context(tc.tile_pool(name="w", bufs=1))
            wq_sb = wpool.tile([128, NH, KT, 128], BF16, name="wq")
            nc.sync.dma_start(wq_sb[:], wq[:, :, :, :])
            wk_sb = wpool.tile([128, NH, KT, 128], BF16, name="wk")
            nc.sync.dma_start(wk_sb[:], wk[:, :, :, :])
            wv_sb = wpool.tile([128, KT, OC], BF16, name="wv")
            nc.sync.dma_start(wv_sb[:], wv[:, :, :])

            xpool = p1.enter_context(tc.tile_pool(name="xg", bufs=2))
            cpool = p1.enter_context(tc.tile_pool(name="csg", bufs=2))
            tmp = p1.enter_context(tc.tile_pool(name="tmp", bufs=8))
            st = p1.enter_context(tc.tile_pool(name="st", bufs=6))
            ps1 = p1.enter_context(tc.tile_pool(name="ps1", bufs=6, space="PSUM"))

            for g in range(NG):
                b = g // (NG // B)
                pos0 = (g % (NG // B)) * TG          # position within batch
                xg = xpool.tile([128, KT, TG], BF16, name="xg")
                nc.sync.dma_start(
                    xg[:, :, :],
                    xT[:, :, g * TG:(g + 1) * TG].transpose([1, 0, 2]))
                csg = cpool.tile([128, TG], F32, name="csg")
                nc.sync.dma_start(csg[:], cs[:, pos0:pos0 + TG])
                sng = cpool.tile([128, TG], F32, name="sng")
                nc.sync.dma_start(sng[:], sn[:, pos0:pos0 + TG])

                # Q and K with fused RoPE
                for wsb, sp in ((wq_sb, qsp[b]), (wk_sb, ksp[b])):
                    for pr in range(NH // 2):
                        p_re = ps1.tile([128, TG], F32, name="ps1")
                        p_im = ps1.tile([128, TG], F32, name="ps1")
                        for kt in range(KT):
                            nc.tensor.matmul(p_re[:], wsb[:, 2 * pr, kt, :],
                                             xg[:, kt, :],
                                             start=(kt == 0), stop=(kt == KT - 1))
                        for kt in range(KT):
                            nc.tensor.matmul(p_im[:], wsb[:, 2 * pr + 1, kt, :],
                                             xg[:, kt, :],
                                             start=(kt == 0), stop=(kt == KT - 1))
                        t1 = tmp.tile([128, TG], F32, name="tmp")
                        t2 = tmp.tile([128, TG], F32, name="tmp")
                        t3 = tmp.tile([128, TG], F32, name="tmp")
                        t4 = tmp.tile([128, TG], F32, name="tmp")
                        o_re = st.tile([128, TG], BF16, name="st")
                        o_im = st.tile([128, TG], BF16, name="st")
                        nc.vector.tensor_mul(t1[:], p_re[:], csg[:])
                        nc.vector.tensor_mul(t2[:], p_im[:], sng[:])
                        nc.vector.tensor_sub(o_re[:], t1[:], t2[:])
                        nc.vector.tensor_mul(t3[:], p_re[:], sng[:])
                        nc.vector.tensor_mul(t4[:], p_im[:], csg[:])
                        nc.vector.tensor_add(o_im[:], t3[:], t4[:])
                        ha, hb = 2 * pr, 2 * pr + 1
                        nc.gpsimd.dma_start(sp[ha, 0:64, pos0:pos0 + TG],
                                            o_re[0:64, :])
                        nc.gpsimd.dma_start(sp[hb, 0:64, pos0:pos0 + TG],
                                            o_re[64:128, :])
                        nc.gpsimd.dma_start(sp[ha, 64:128, pos0:pos0 + TG],
                                            o_im[0:64, :])
                        nc.gpsimd.dma_start(sp[hb, 64:128, pos0:pos0 + TG],
                                            o_im[64:128, :])

                # V (layout [t, oc])
                for sub in range(TG // 128):
                    pv = ps1.tile([128, OC], F32, name="ps1")
                    for kt in range(KT):
                        nc.tensor.matmul(pv[:], xg[:, kt, sub * 128:(sub + 1) * 128],
                                         wv_sb[:, kt, :],
                                         start=(kt == 0), stop=(kt == KT - 1))
                    vo = st.tile([128, OC], BF16, name="st")
                    nc.scalar.copy(vo[:], pv[:])
                    tt = pos0 // 128 + sub
                    nc.gpsimd.dma_start(vsp[b][tt, :, :], vo[:])

        # ================= Phase 2: attention (scores/softmax/PV) ==========
        with ExitStack() as p2:
            qk = p2.enter_context(tc.tile_pool(name="qk", bufs=2))
            vbp = p2.enter_context(tc.tile_pool(name="vb", bufs=2))
            ep = p2.enter_context(tc.tile_pool(name="ep", bufs=3))
            rap = p2.enter_context(tc.tile_pool(name="racc", bufs=2))
            rcp = p2.enter_context(tc.tile_pool(name="rcast", bufs=2))
            rc = p2.enter_context(tc.tile_pool(name="rc", bufs=2))
            ao = p2.enter_context(tc.tile_pool(name="ao", bufs=3))
            ps_s = p2.enter_context(tc.tile_pool(name="ps_s", bufs=2, space="PSUM"))
            ps_pv = p2.enter_context(tc.tile_pool(name="ps_pv", bufs=2, space="PSUM"))
            ps_r = p2.enter_context(tc.tile_pool(name="ps_r", bufs=1, space="PSUM"))

            for b in range(B):
                vb = vbp.tile([128, LT, OC], BF16, name="vb")
                nc.sync.dma_start(vb[:, :, :],
                                  vsp[b].ap().transpose([1, 0, 2]))
                for h in range(NH):
                    q_sb = qk.tile([128, L], BF16, name="q")
                    nc.sync.dma_start(q_sb[:], qsp[b][h, :, :])
                    k_sb = qk.tile([128, L], BF16, name="k")
                    nc.sync.dma_start(k_sb[:], ksp[b][h, :, :])
                    for half in range(2):
                        q0 = half * 1024
                        pvs = [ps_pv.tile([128, 512], F32, name="ps_pv")
                               for _ in range(2)]
                        racc = rap.tile([128, 1024], F32, name="racc")
                        for kt in range(LT):
                            s_ps = ps_s.tile([128, 1024], F32, name="ps_s")
                            nc.tensor.matmul(s_ps[:, 0:512],
                                             k_sb[:, kt * 128:(kt + 1) * 128],
                                             q_sb[:, q0:q0 + 512],
                                             start=True, stop=True)
                            nc.tensor.matmul(s_ps[:, 512:1024],
                                             k_sb[:, kt * 128:(kt + 1) * 128],
                                             q_sb[:, q0 + 512:q0 + 1024],
                                             start=True, stop=True)
                            e_t = ep.tile([128, 1024], BF16, name="ep")
                            nc.scalar.activation(e_t[:], s_ps[:], EXP_F, scale=SCALE)
                            first, last = (kt == 0), (kt == LT - 1)
                            for c in range(2):
                                nc.tensor.matmul(pvs[c][:],
                                                 vb[:, kt, h * 128:(h + 1) * 128],
                                                 e_t[:, c * 512:(c + 1) * 512],
                                                 start=first, stop=last)
                            # row-sum partials on DVE (partition-wise)
                            if first:
                                nc.vector.tensor_copy(racc[:], e_t[:])
                            else:
                                nc.vector.tensor_add(racc[:], racc[:], e_t[:])
                        # reduce racc over partitions via one ones-matmul
                        rcast = rcp.tile([128, 1024], BF16, name="rcast")
                        nc.vector.tensor_copy(rcast[:], racc[:])
                        r_ps = ps_r.tile([128, 1024], F32, name="ps_r")
                        for c in range(2):
                            nc.tensor.matmul(r_ps[:, c * 512:(c + 1) * 512],
                                             ones_sb[:],
                                             rcast[:, c * 512:(c + 1) * 512],
                                             start=True, stop=True)
                        rec = rc.tile([128, 1024], F32, name="rc")
                        nc.vector.reciprocal_approx_fast(out=rec[:], in_=r_ps[:])
                        for c in range(2):
                            at = ao.tile([128, 512], BF16, name="ao")
                            nc.vector.tensor_mul(at[:], pvs[c][:],
                                                 rec[:, c * 512:(c + 1) * 512])
                            ci = half * 2 + c
                            nc.gpsimd.dma_start(
                                a2a_in[b][h][2 * ci, :, :], at[:, 0:SHB])
                            nc.gpsimd.dma_start(
                                a2a_in[b][h][2 * ci + 1, :, :], at[:, SHB:2 * SHB])
                    nc.gpsimd.collective_compute(
                        "AllToAll", mybir.AluOpType.bypass,
                        replica_groups=[list(range(NCORES))],
                        ins=[a2a_in[b][h].ap().opt()],
                        outs=[a2a_out[b][h].ap().opt()],
                    )

        # ================= Phase 3: o_proj on this core's token shard ======
        with ExitStack() as p3:
            rhp = p3.enter_context(tc.tile_pool(name="rh", bufs=1))
            wop = p3.enter_context(tc.tile_pool(name="wo", bufs=3))
            osb = p3.enter_context(tc.tile_pool(name="osb", bufs=4))
            ps_o = p3.enter_context(tc.tile_pool(name="ps_o", bufs=2, space="PSUM"))
            rh = rhp.tile([128, KT, SH], BF16, name="rh")
            rh4 = rh[:].rearrange("p (j f) t -> p j f t", f=4)   # [128,8,4,SH]
            for b in range(B):
                for h in range(NH):
                    # global kt = 4*j + h
                    nc.sync.dma_start(
                        rh4[:, :, h, b * SHB:(b + 1) * SHB],
                        a2a_out[b][h].ap().transpose([1, 0, 2]))
            # kt order h-major so the first chain only needs the h=0 exchanges
            kt_order = [4 * j + hh for hh in range(NH) for j in range(NCORES)]
            for ot in range(D // 128):
                wot = wop.tile([128, KT, 128], BF16, name="wo")
                nc.sync.dma_start(wot[:], wo[ot, :, :, :])
                po = ps_o.tile([128, SH], F32, name="ps_o")
                for i, kt in enumerate(kt_order):
                    nc.tensor.matmul(po[:], wot[:, kt, :], rh[:, kt, :],
                                     start=(i == 0), stop=(i == KT - 1))
                o_sb = osb.tile([128, SH], F32, name="osb")
                nc.scalar.copy(o_sb[:], po[:])
                nc.gpsimd.dma_start(out[ot * 128:(ot + 1) * 128, :], o_sb[:])

    nc.compile()
    return nc


def _qk_row_perm():
    # local row order: [h0re|h1re],[h0im|h1im],[h2re|h3re],[h2im|h3im]
    rows = []
    for pr in range(NH // 2):
        ha, hb = 2 * pr, 2 * pr + 1
        rows += [ha * HD + 2 * i for i in range(HD // 2)]
        rows += [hb * HD + 2 * i for i in range(HD // 2)]
        rows += [ha * HD + 2 * i + 1 for i in range(HD // 2)]
        rows += [hb * HD + 2 * i + 1 for i in range(HD // 2)]
    return np.array(rows)


def _prep_inputs(x, freqs_cos, freqs_sin, Wq, Wk, Wv, Wo):
    x = np.asarray(x, np.float32).reshape(T, D)
    Wq, Wk, Wv, Wo = (np.asarray(w, np.float32) for w in (Wq, Wk, Wv, Wo))
    fc = np.asarray(freqs_cos, np.float32)
    fs = np.asarray(freqs_sin, np.float32)

    # shared tensors
    xT = np.ascontiguousarray(
        x.reshape(T, KT, 128).transpose(1, 2, 0)).astype(NPBF16)        # [KT,128,T]
    woh = np.ascontiguousarray(
        Wo.reshape(D // 128, 128, KT, 128).transpose(0, 3, 2, 1)).astype(NPBF16)
    csh = np.ascontiguousarray(np.concatenate([fc.T, fc.T], 0))          # [128,L]
    snh = np.ascontiguousarray(np.concatenate([fs.T, fs.T], 0))
    ones = np.ones([128, 128], NPBF16)

    perm = _qk_row_perm()
    in_maps = []
    for i in range(NCORES):
        rows = slice(OC * i, OC * (i + 1))
        wqi = Wq[rows][perm]                                             # [512, D]
        wki = Wk[rows][perm]
        wqh = np.ascontiguousarray(
            wqi.reshape(NH, 128, KT, 128).transpose(3, 0, 2, 1)).astype(NPBF16)
        wkh = np.ascontiguousarray(
            wki.reshape(NH, 128, KT, 128).transpose(3, 0, 2, 1)).astype(NPBF16)
        wvh = np.ascontiguousarray(
            Wv[rows].reshape(OC, KT, 128).transpose(2, 1, 0)).astype(NPBF16)
        in_maps.append({
            "xT": xT, "wq": wqh, "wk": wkh, "wv": wvh, "wo": woh,
            "cs": csh, "sn": snh, "ones": ones,
        })
    return in_maps


_NC_CACHE = None


def _get_nc():
    global _NC_CACHE
    if _NC_CACHE is None:
        _NC_CACHE = build_nc()
    return _NC_CACHE


def _run(in_maps, trace=False):
    nc = _get_nc()
    res = bass_utils.run_bass_kernel_spmd(
        nc, in_maps, core_ids=list(range(NCORES)), trace=trace)
    return res


def _assemble(results):
    out = np.empty((B, L, D), np.float32)
    for i in range(NCORES):
        o = results[i]["out"]                       # [D, SH] f32
        for b in range(B):
            out[b, SHB * i:SHB * (i + 1), :] = o[:, b * SHB:(b + 1) * SHB].T
    return out


def kernel(x, freqs_cos, freqs_sin, Wq, Wk, Wv, Wo):
    in_maps = _prep_inputs(x, freqs_cos, freqs_sin, Wq, Wk, Wv, Wo)
    res = _run(in_maps, trace=False)
    return _assemble(res.results)


# revision 10
# speedup vs baseline: 1.0919x; 1.0173x over previous
"""Distributed multi-head attention (B=2, L=2048, D=4096, H=32) on 8 TRN2 NeuronCores.

Strategy: tensor-parallel over heads (4 heads/core) for QKV+attention, then an
AllToAll that trades head-dims for token-slices so o_proj is token-sharded
(each core computes out[:, its 512 tokens] with the full Wo) — the AllToAll
moves 4 MB/core instead of the 64 MB/core an output AllReduce would.

All matmuls run in bf16 on the TensorEngine (f32 PSUM accumulation).
Host-side prep: transpose/permute/tile weights and x into DMA-friendly
partition-major layouts, pre-cast to bf16. Host post: concatenate the 8
token-shards and transpose. Verified rel-err vs f32 reference ~6e-3.

RoPE trick: Q/K output columns are permuted host-side (per head: even dims
then odd dims, pairs of heads interleaved into 128-row tiles) so the rotation
becomes full-width [128, t] vector ops with no partition-pair shuffles.

Perf notes (iter2): all multi-tile loads are single dma_starts (one semaphore
wait instead of one per matmul — per-MM issue measured 266ns vs the 225ns
floor when every kt-tile MM waits its own DMA sem); o_proj is one N=512 pass
over both batches; the AllToAll is split 4 ways (per batch x head-pair) so
only the last ~1MB exchange is exposed; stores go on the GpSimd DMA queue,
loads on Sync.
"""

import sys

if "/opt/trn_rl_repo" not in sys.path:
    sys.path.insert(0, "/opt/trn_rl_repo")

from contextlib import ExitStack

import ml_dtypes
import numpy as np

import concourse.bass as bass
import concourse.tile as tile
from concourse import bacc, mybir
from concourse import bass_utils

BF16 = mybir.dt.bfloat16
F32 = mybir.dt.float32
NPBF16 = ml_dtypes.bfloat16

NCORES = 8
B, L, D, H, HD = 2, 2048, 4096, 32, 128
T = B * L              # 4096 global tokens
NH = H // NCORES       # 4 heads per core
OC = NH * HD           # 512 projection dims per core
KT = D // 128          # 32 contraction tiles over D
LT = L // 128          # 16 key tiles per batch
TG = 512               # phase-1 token-group width
NG = T // TG           # 8 groups
SH = T // NCORES       # 512 output tokens per core
SHB = SH // B          # 256 per batch
SCALE = 1.0 / float(np.sqrt(HD))

EXP_F = mybir.ActivationFunctionType.Exp


def build_nc():
    nc = bacc.Bacc("TRN2", target_bir_lowering=False, debug=False,
                   num_devices=NCORES)

    # ---- I/O (per-core shards, host-pretiled, bf16) ----
    xT = nc.dram_tensor("xT", [KT, 128, T], BF16, kind="ExternalInput")
    wq = nc.dram_tensor("wq", [128, NH, KT, 128], BF16, kind="ExternalInput")
    wk = nc.dram_tensor("wk", [128, NH, KT, 128], BF16, kind="ExternalInput")
    wv = nc.dram_tensor("wv", [128, KT, OC], BF16, kind="ExternalInput")
    wo = nc.dram_tensor("wo", [D // 128, 128, KT, 128], BF16, kind="ExternalInput")
    cs = nc.dram_tensor("cs", [128, L], F32, kind="ExternalInput")
    sn = nc.dram_tensor("sn", [128, L], F32, kind="ExternalInput")
    ones = nc.dram_tensor("ones", [128, 128], BF16, kind="ExternalInput")
    out = nc.dram_tensor("out", [D, SH], F32, kind="ExternalOutput")

    # ---- internal DRAM (spills + collective bounce) ----
    qsp = [nc.dram_tensor(f"qsp{b}", [NH, 128, L], BF16) for b in range(B)]
    ksp = [nc.dram_tensor(f"ksp{b}", [NH, 128, L], BF16) for b in range(B)]
    vsp = [nc.dram_tensor(f"vsp{b}", [LT, 128, OC], BF16) for b in range(B)]
    # AllToAll split by (batch, head): [shard, HD dims, SHB tokens]
    a2a_in = [[nc.dram_tensor(f"a2ai{b}_{h}", [NCORES, HD, SHB], BF16)
               for h in range(NH)] for b in range(B)]
    a2a_out = [[nc.dram_tensor(f"a2ao{b}_{h}", [NCORES, HD, SHB], BF16)
                for h in range(NH)] for b in range(B)]

    with tile.TileContext(nc) as tc, ExitStack() as ctx:
        singles = ctx.enter_context(tc.tile_pool(name="singles", bufs=1))
        ones_sb = singles.tile([128, 128], BF16, name="ones")
        nc.sync.dma_start(ones_sb[:], ones[:, :])

        # ================= Phase 1: QKV projections + RoPE =================
        with ExitStack() as p1:
            wpool = p1.enter_context(tc.tile_pool(name="w", bufs=1))
            wq_sb = wpool.tile([128, NH, KT, 128], BF16, name="wq")
            nc.sync.dma_start(wq_sb[:], wq[:, :, :, :])
            wk_sb = wpool.tile([128, NH, KT, 128], BF16, name="wk")
            nc.sync.dma_start(wk_sb[:], wk[:, :, :, :])
            wv_sb = wpool.tile([128, KT, OC], BF16, name="wv")
            nc.sync.dma_start(wv_sb[:], wv[:, :, :])

            xpool = p1.enter_context(tc.tile_pool(name="xg", bufs=2))
            cpool = p1.enter_context(tc.tile_pool(name="csg", bufs=2))
            tmp = p1.enter_context(tc.tile_pool(name="tmp", bufs=8))
            st = p1.enter_context(tc.tile_pool(name="st", bufs=6))
            ps1 = p1.enter_context(tc.tile_pool(name="ps1", bufs=6, space="PSUM"))

            for g in range(NG):
                b = g // (NG // B)
                pos0 = (g % (NG // B)) * TG          # position within batch
                xg = xpool.tile([128, KT, TG], BF16, name="xg")
                nc.sync.dma_start(
                    xg[:, :, :],
                    xT[:, :, g * TG:(g + 1) * TG].transpose([1, 0, 2]))
                csg = cpool.tile([128, TG], F32, name="csg")
                nc.sync.dma_start(csg[:], cs[:, pos0:pos0 + TG])
                sng = cpool.tile([128, TG], F32, name="sng")
                nc.sync.dma_start(sng[:], sn[:, pos0:pos0 + TG])

                # Q and K with fused RoPE
                for wsb, sp in ((wq_sb, qsp[b]), (wk_sb, ksp[b])):
                    for pr in range(NH // 2):
                        p_re = ps1.tile([128, TG], F32, name="ps1")
                        p_im = ps1.tile([128, TG], F32, name="ps1")
                        for kt in range(KT):
                            nc.tensor.matmul(p_re[:], wsb[:, 2 * pr, kt, :],
                                             xg[:, kt, :],
                                             start=(kt == 0), stop=(kt == KT - 1))
                        for kt in range(KT):
                            nc.tensor.matmul(p_im[:], wsb[:, 2 * pr + 1, kt, :],
                                             xg[:, kt, :],
                                             start=(kt == 0), stop=(kt == KT - 1))
                        t1 = tmp.tile([128, TG], F32, name="tmp")
                        t2 = tmp.tile([128, TG], F32, name="tmp")
                        t3 = tmp.tile([128, TG], F32, name="tmp")
                        t4 = tmp.tile([128, TG], F32, name="tmp")
                        o_re = st.tile([128, TG], BF16, name="st")
                        o_im = st.tile([128, TG], BF16, name="st")
                        nc.vector.tensor_mul(t1[:], p_re[:], csg[:])
                        nc.vector.tensor_mul(t2[:], p_im[:], sng[:])
                        nc.vector.tensor_sub(o_re[:], t1[:], t2[:])
                        nc.vector.tensor_mul(t3[:], p_re[:], sng[:])
                        nc.vector.tensor_mul(t4[:], p_im[:], csg[:])
                        nc.vector.tensor_add(o_im[:], t3[:], t4[:])
                        ha, hb = 2 * pr, 2 * pr + 1
                        nc.gpsimd.dma_start(sp[ha, 0:64, pos0:pos0 + TG],
                                            o_re[0:64, :])
                        nc.gpsimd.dma_start(sp[hb, 0:64, pos0:pos0 + TG],
                                            o_re[64:128, :])
                        nc.gpsimd.dma_start(sp[ha, 64:128, pos0:pos0 + TG],
                                            o_im[0:64, :])
                        nc.gpsimd.dma_start(sp[hb, 64:128, pos0:pos0 + TG],
                                            o_im[64:128, :])

                # V (layout [t, oc])
                for sub in range(TG // 128):
                    pv = ps1.tile([128, OC], F32, name="ps1")
                    for kt in range(KT):
                        nc.tensor.matmul(pv[:], xg[:, kt, sub * 128:(sub + 1) * 128],
                                         wv_sb[:, kt, :],
                                         start=(kt == 0), stop=(kt == KT - 1))
                    vo = st.tile([128, OC], BF16, name="st")
                    nc.scalar.copy(vo[:], pv[:])
                    tt = pos0 // 128 + sub
                    nc.gpsimd.dma_start(vsp[b][tt, :, :], vo[:])

        # ============ Phase 2+3: attention, AllToAll, o_proj (overlapped) =====
        with ExitStack() as p2:
            qk = p2.enter_context(tc.tile_pool(name="qk", bufs=2))
            vbp = p2.enter_context(tc.tile_pool(name="vb", bufs=1))
            ep = p2.enter_context(tc.tile_pool(name="ep", bufs=3))
            trp = p2.enter_context(tc.tile_pool(name="tr", bufs=6))
            rc = p2.enter_context(tc.tile_pool(name="rc", bufs=2))
            ao = p2.enter_context(tc.tile_pool(name="ao", bufs=3))
            rhp = p2.enter_context(tc.tile_pool(name="rh", bufs=1))
            wop = p2.enter_context(tc.tile_pool(name="wo", bufs=3))
            oac = p2.enter_context(tc.tile_pool(name="oac", bufs=1))
            osb = p2.enter_context(tc.tile_pool(name="osb", bufs=4))
            ps_s = p2.enter_context(tc.tile_pool(name="ps_s", bufs=2, space="PSUM"))
            ps_pv = p2.enter_context(tc.tile_pool(name="ps_pv", bufs=2, space="PSUM"))
            ps_o = p2.enter_context(tc.tile_pool(name="ps_o", bufs=2, space="PSUM"))

            for b in range(B):
                vb = vbp.tile([128, LT, OC], BF16, name="vb")
                nc.sync.dma_start(vb[:, :, :],
                                  vsp[b].ap().transpose([1, 0, 2]))
                for h in range(NH):
                    q_sb = qk.tile([128, L], BF16, name="q")
                    nc.sync.dma_start(q_sb[:], qsp[b][h, :, :])
                    k_sb = qk.tile([128, L], BF16, name="k")
                    nc.sync.dma_start(k_sb[:], ksp[b][h, :, :])
                    for half in range(2):
                        q0 = half * 1024
                        pvs = [ps_pv.tile([128, 512], F32, name="ps_pv")
                               for _ in range(2)]
                        tree = []          # bf16 pairwise row-sum tree
                        for kt in range(LT):
                            s_ps = ps_s.tile([128, 1024], F32, name="ps_s")
                            nc.tensor.matmul(s_ps[:, 0:512],
                                             k_sb[:, kt * 128:(kt + 1) * 128],
                                             q_sb[:, q0:q0 + 512],
                                             start=True, stop=True)
                            nc.tensor.matmul(s_ps[:, 512:1024],
                                             k_sb[:, kt * 128:(kt + 1) * 128],
                                             q_sb[:, q0 + 512:q0 + 1024],
                                             start=True, stop=True)
                            e_t = ep.tile([128, 1024], BF16, name="ep")
                            nc.scalar.activation(e_t[:], s_ps[:], EXP_F, scale=SCALE)
                            first, last = (kt == 0), (kt == LT - 1)
                            for c in range(2):
                                nc.tensor.matmul(pvs[c][:],
                                                 vb[:, kt, h * 128:(h + 1) * 128],
                                                 e_t[:, c * 512:(c + 1) * 512],
                                                 start=first, stop=last)
                            node = (0, e_t)
                            while tree and tree[-1][0] == node[0]:
                                prev = tree.pop()
                                nt = trp.tile([128, 1024], BF16, name="tr")
                                nc.vector.tensor_add(nt[:], prev[1][:], node[1][:])
                                node = (node[0] + 1, nt)
                            tree.append(node)
                        assert len(tree) == 1
                        root = tree[0][1]
                        # reduce the 128 partition partials in one f32 matmul
                        r_ps = ps_s.tile([128, 1024], F32, name="ps_s")
                        for c in range(2):
                            nc.tensor.matmul(r_ps[:, c * 512:(c + 1) * 512],
                                             ones_sb[:],
                                             root[:, c * 512:(c + 1) * 512],
                                             start=True, stop=True)
                        rec = rc.tile([128, 1024], F32, name="rc")
                        nc.vector.reciprocal_approx_fast(out=rec[:], in_=r_ps[:])
                        for c in range(2):
                            at = ao.tile([128, 512], BF16, name="ao")
                            nc.vector.tensor_mul(at[:], pvs[c][:],
                                                 rec[:, c * 512:(c + 1) * 512])
                            ci = half * 2 + c
                            nc.gpsimd.dma_start(
                                a2a_in[b][h][2 * ci, :, :], at[:, 0:SHB])
                            nc.gpsimd.dma_start(
                                a2a_in[b][h][2 * ci + 1, :, :], at[:, SHB:2 * SHB])
                    nc.gpsimd.collective_compute(
                        "AllToAll", mybir.AluOpType.bypass,
                        replica_groups=[list(range(NCORES))],
                        ins=[a2a_in[b][h].ap().opt()],
                        outs=[a2a_out[b][h].ap().opt()],
                    )

            # ---- o_proj: two passes over head-groups, SBUF f32 accumulation
            rh = rhp.tile([128, KT, SH], BF16, name="rh")
            rh4 = rh[:].rearrange("p (j f) t -> p j f t", f=4)   # [128,8,4,SH]
            for b in range(B):
                for h in range(NH):
                    # global kt = 4*j + h
                    nc.sync.dma_start(
                        rh4[:, :, h, b * SHB:(b + 1) * SHB],
                        a2a_out[b][h].ap().transpose([1, 0, 2]))
            out_acc = oac.tile([128, D // 128, SH], F32, name="oac")
            # wo dram [ot, p, (j f), o] with f=4; halves f 0:2 / 2:4 are contiguous
            wo4 = wo.ap().rearrange("ot p (j f) o -> ot p j (f o)", f=4)
            for pss in range(2):
                for ot in range(D // 128):
                    wot = wop.tile([128, NCORES, 256], BF16, name="wo")
                    nc.sync.dma_start(
                        wot[:],
                        wo4[ot, :, :, pss * 256:(pss + 1) * 256])
                    po = ps_o.tile([128, SH], F32, name="ps_o")
                    i = 0
                    for hh in (2 * pss, 2 * pss + 1):
                        for j in range(NCORES):
                            nc.tensor.matmul(po[:], wot[:, j, (hh % 2) * 128:(hh % 2) * 128 + 128],
                                             rh[:, 4 * j + hh, :],
                                             start=(i == 0), stop=(i == 15))
                            i += 1
                    if pss == 0:
                        nc.scalar.copy(out_acc[:, ot, :], po[:])
                    else:
                        o_sb = osb.tile([128, SH], F32, name="osb")
                        nc.vector.tensor_add(o_sb[:], po[:], out_acc[:, ot, :])
                        nc.gpsimd.dma_start(out[ot * 128:(ot + 1) * 128, :], o_sb[:])

    nc.compile()
    return nc


def _qk_row_perm():
    # local row order: [h0re|h1re],[h0im|h1im],[h2re|h3re],[h2im|h3im]
    rows = []
    for pr in range(NH // 2):
        ha, hb = 2 * pr, 2 * pr + 1
        rows += [ha * HD + 2 * i for i in range(HD // 2)]
        rows += [hb * HD + 2 * i for i in range(HD // 2)]
        rows += [ha * HD + 2 * i + 1 for i in range(HD // 2)]
        rows += [hb * HD + 2 * i + 1 for i in range(HD // 2)]
    return np.array(rows)


def _prep_inputs(x, freqs_cos, freqs_sin, Wq, Wk, Wv, Wo):
    x = np.asarray(x, np.float32).reshape(T, D)
    Wq, Wk, Wv, Wo = (np.asarray(w, np.float32) for w in (Wq, Wk, Wv, Wo))
    fc = np.asarray(freqs_cos, np.float32)
    fs = np.asarray(freqs_sin, np.float32)

    # shared tensors
    xT = np.ascontiguousarray(
        x.reshape(T, KT, 128).transpose(1, 2, 0)).astype(NPBF16)        # [KT,128,T]
    woh = np.ascontiguousarray(
        Wo.reshape(D // 128, 128, KT, 128).transpose(0, 3, 2, 1)).astype(NPBF16)
    csh = np.ascontiguousarray(np.concatenate([fc.T, fc.T], 0))          # [128,L]
    snh = np.ascontiguousarray(np.concatenate([fs.T, fs.T], 0))
    ones = np.ones([128, 128], NPBF16)

    perm = _qk_row_perm()
    in_maps = []
    for i in range(NCORES):
        rows = slice(OC * i, OC * (i + 1))
        wqi = Wq[rows][perm]                                             # [512, D]
        wki = Wk[rows][perm]
        wqh = np.ascontiguousarray(
            wqi.reshape(NH, 128, KT, 128).transpose(3, 0, 2, 1)).astype(NPBF16)
        wkh = np.ascontiguousarray(
            wki.reshape(NH, 128, KT, 128).transpose(3, 0, 2, 1)).astype(NPBF16)
        wvh = np.ascontiguousarray(
            Wv[rows].reshape(OC, KT, 128).transpose(2, 1, 0)).astype(NPBF16)
        in_maps.append({
            "xT": xT, "wq": wqh, "wk": wkh, "wv": wvh, "wo": woh,
            "cs": csh, "sn": snh, "ones": ones,
        })
    return in_maps


_NC_CACHE = None


def _get_nc():
    global _NC_CACHE
    if _NC_CACHE is None:
        _NC_CACHE = build_nc()
    return _NC_CACHE


def _run(in_maps, trace=False):
    nc = _get_nc()
    res = bass_utils.run_bass_kernel_spmd(
        nc, in_maps, core_ids=list(range(NCORES)), trace=trace)
    return res


def _assemble(results):
    out = np.empty((B, L, D), np.float32)
    for i in range(NCORES):
        o = results[i]["out"]                       # [D, SH] f32
        for b in range(B):
            out[b, SHB * i:SHB * (i + 1), :] = o[:, b * SHB:(b + 1) * SHB].T
    return out


def kernel(x, freqs_cos, freqs_sin, Wq, Wk, Wv, Wo):
    in_maps = _prep_inputs(x, freqs_cos, freqs_sin, Wq, Wk, Wv, Wo)
    res = _run(in_maps, trace=False)
    return _assemble(res.results)


# revision 12
# speedup vs baseline: 1.1267x; 1.0318x over previous
"""Distributed multi-head attention (B=2, L=2048, D=4096, H=32) on 8 TRN2 NeuronCores.

Strategy: tensor-parallel over heads (4 heads/core) for QKV+attention, then an
AllToAll that trades head-dims for token-slices so o_proj is token-sharded
(each core computes out[:, its 512 tokens] with the full Wo) — the AllToAll
moves 4 MB/core instead of the 64 MB/core an output AllReduce would.

All matmuls run in bf16 on the TensorEngine (f32 PSUM accumulation).
Host-side prep: transpose/permute/tile weights and x into DMA-friendly
partition-major layouts, pre-cast to bf16. Host post: concatenate the 8
token-shards and transpose. Verified rel-err vs f32 reference ~6e-3.

RoPE trick: Q/K output columns are permuted host-side (per head: even dims
then odd dims, pairs of heads interleaved into 128-row tiles) so the rotation
becomes full-width [128, t] vector ops with no partition-pair shuffles.

Perf notes: all multi-tile loads are single dma_starts; stores go on the
GpSimd DMA queue, loads on Sync.  The softmax row-sum is a bf16 pairwise
tree on the VectorEngine (533ns/add in 2x mode; the depth-4 bf16 error
averages out in the final f32 ones-matmul partition-reduce, measured
<1e-3 on the row sums) so the TensorEngine only runs scores+PV in the
attention inner loop.  The AllToAll is split 8 ways (per batch x head) and
issued as each head finishes, so only the last ~0.5MB exchange is exposed.
o_proj runs as two head-group passes accumulating in SBUF f32 — the first
pass's matmuls fill TensorEngine bubbles during the ACT-bound attention
phase, the second pass needs only the last exchanges.  Measured
1.57ms on silicon (PE-array busy ~86%, QKV phase >99%).
"""

import sys

if "/opt/trn_rl_repo" not in sys.path:
    sys.path.insert(0, "/opt/trn_rl_repo")

from contextlib import ExitStack

import ml_dtypes
import numpy as np

import concourse.bass as bass
import concourse.tile as tile
from concourse import bacc, mybir
from concourse import bass_utils

BF16 = mybir.dt.bfloat16
F32 = mybir.dt.float32
NPBF16 = ml_dtypes.bfloat16

NCORES = 8
B, L, D, H, HD = 2, 2048, 4096, 32, 128
T = B * L              # 4096 global tokens
NH = H // NCORES       # 4 heads per core
OC = NH * HD           # 512 projection dims per core
KT = D // 128          # 32 contraction tiles over D
LT = L // 128          # 16 key tiles per batch
TG = 512               # phase-1 token-group width
NG = T // TG           # 8 groups
SH = T // NCORES       # 512 output tokens per core
SHB = SH // B          # 256 per batch
SCALE = 1.0 / float(np.sqrt(HD))

EXP_F = mybir.ActivationFunctionType.Exp


def build_nc():
    nc = bacc.Bacc("TRN2", target_bir_lowering=False, debug=False,
                   num_devices=NCORES)

    # ---- I/O (per-core shards, host-pretiled, bf16) ----
    xT = nc.dram_tensor("xT", [KT, 128, T], BF16, kind="ExternalInput")
    wq = nc.dram_tensor("wq", [128, NH, KT, 128], BF16, kind="ExternalInput")
    wk = nc.dram_tensor("wk", [128, NH, KT, 128], BF16, kind="ExternalInput")
    wv = nc.dram_tensor("wv", [128, KT, OC], BF16, kind="ExternalInput")
    wo = nc.dram_tensor("wo", [D // 128, 128, KT, 128], BF16, kind="ExternalInput")
    cs = nc.dram_tensor("cs", [128, L], F32, kind="ExternalInput")
    sn = nc.dram_tensor("sn", [128, L], F32, kind="ExternalInput")
    ones = nc.dram_tensor("ones", [128, 128], BF16, kind="ExternalInput")
    out = nc.dram_tensor("out", [D, SH], F32, kind="ExternalOutput")

    # ---- internal DRAM (spills + collective bounce) ----
    qsp = [nc.dram_tensor(f"qsp{b}", [NH, 128, L], BF16) for b in range(B)]
    ksp = [nc.dram_tensor(f"ksp{b}", [NH, 128, L], BF16) for b in range(B)]
    vsp = [nc.dram_tensor(f"vsp{b}", [LT, 128, OC], BF16) for b in range(B)]
    # AllToAll split by (batch, head): [shard, HD dims, SHB tokens]
    a2a_in = [[nc.dram_tensor(f"a2ai{b}_{h}", [NCORES, HD, SHB], BF16)
               for h in range(NH)] for b in range(B)]
    a2a_out = [[nc.dram_tensor(f"a2ao{b}_{h}", [NCORES, HD, SHB], BF16)
                for h in range(NH)] for b in range(B)]

    with tile.TileContext(nc) as tc, ExitStack() as ctx:
        singles = ctx.enter_context(tc.tile_pool(name="singles", bufs=1))
        ones_sb = singles.tile([128, 128], BF16, name="ones")
        nc.sync.dma_start(ones_sb[:], ones[:, :])

        # ================= Phase 1: QKV projections + RoPE =================
        with ExitStack() as p1:
            wpool = p1.enter_context(tc.tile_pool(name="w", bufs=1))
            wq_sb = wpool.tile([128, NH, KT, 128], BF16, name="wq")
            nc.sync.dma_start(wq_sb[:], wq[:, :, :, :])
            wk_sb = wpool.tile([128, NH, KT, 128], BF16, name="wk")
            nc.sync.dma_start(wk_sb[:], wk[:, :, :, :])
            wv_sb = wpool.tile([128, KT, OC], BF16, name="wv")
            nc.sync.dma_start(wv_sb[:], wv[:, :, :])

            xpool = p1.enter_context(tc.tile_pool(name="xg", bufs=2))
            cpool = p1.enter_context(tc.tile_pool(name="csg", bufs=2))
            tmp = p1.enter_context(tc.tile_pool(name="tmp", bufs=8))
            st = p1.enter_context(tc.tile_pool(name="st", bufs=6))
            ps1 = p1.enter_context(tc.tile_pool(name="ps1", bufs=6, space="PSUM"))

            for g in range(NG):
                b = g // (NG // B)
                pos0 = (g % (NG // B)) * TG          # position within batch
                xg = xpool.tile([128, KT, TG], BF16, name="xg")
                nc.sync.dma_start(
                    xg[:, :, :],
                    xT[:, :, g * TG:(g + 1) * TG].transpose([1, 0, 2]))
                csg = cpool.tile([128, TG], F32, name="csg")
                nc.sync.dma_start(csg[:], cs[:, pos0:pos0 + TG])
                sng = cpool.tile([128, TG], F32, name="sng")
                nc.sync.dma_start(sng[:], sn[:, pos0:pos0 + TG])

                # Q and K with fused RoPE
                for wsb, sp in ((wq_sb, qsp[b]), (wk_sb, ksp[b])):
                    for pr in range(NH // 2):
                        p_re = ps1.tile([128, TG], F32, name="ps1")
                        p_im = ps1.tile([128, TG], F32, name="ps1")
                        for kt in range(KT):
                            nc.tensor.matmul(p_re[:], wsb[:, 2 * pr, kt, :],
                                             xg[:, kt, :],
                                             start=(kt == 0), stop=(kt == KT - 1))
                        for kt in range(KT):
                            nc.tensor.matmul(p_im[:], wsb[:, 2 * pr + 1, kt, :],
                                             xg[:, kt, :],
                                             start=(kt == 0), stop=(kt == KT - 1))
                        t1 = tmp.tile([128, TG], F32, name="tmp")
                        t2 = tmp.tile([128, TG], F32, name="tmp")
                        t3 = tmp.tile([128, TG], F32, name="tmp")
                        t4 = tmp.tile([128, TG], F32, name="tmp")
                        o_re = st.tile([128, TG], BF16, name="st")
                        o_im = st.tile([128, TG], BF16, name="st")
                        nc.vector.tensor_mul(t1[:], p_re[:], csg[:])
                        nc.vector.tensor_mul(t2[:], p_im[:], sng[:])
                        nc.vector.tensor_sub(o_re[:], t1[:], t2[:])
                        nc.vector.tensor_mul(t3[:], p_re[:], sng[:])
                        nc.vector.tensor_mul(t4[:], p_im[:], csg[:])
                        nc.vector.tensor_add(o_im[:], t3[:], t4[:])
                        ha, hb = 2 * pr, 2 * pr + 1
                        nc.gpsimd.dma_start(sp[ha, 0:64, pos0:pos0 + TG],
                                            o_re[0:64, :])
                        nc.gpsimd.dma_start(sp[hb, 0:64, pos0:pos0 + TG],
                                            o_re[64:128, :])
                        nc.gpsimd.dma_start(sp[ha, 64:128, pos0:pos0 + TG],
                                            o_im[0:64, :])
                        nc.gpsimd.dma_start(sp[hb, 64:128, pos0:pos0 + TG],
                                            o_im[64:128, :])

                # V (layout [t, oc])
                for sub in range(TG // 128):
                    pv = ps1.tile([128, OC], F32, name="ps1")
                    for kt in range(KT):
                        nc.tensor.matmul(pv[:], xg[:, kt, sub * 128:(sub + 1) * 128],
                                         wv_sb[:, kt, :],
                                         start=(kt == 0), stop=(kt == KT - 1))
                    vo = st.tile([128, OC], BF16, name="st")
                    nc.scalar.copy(vo[:], pv[:])
                    tt = pos0 // 128 + sub
                    nc.gpsimd.dma_start(vsp[b][tt, :, :], vo[:])

        # ============ Phase 2+3: attention, AllToAll, o_proj (overlapped) =====
        with ExitStack() as p2:
            qk = p2.enter_context(tc.tile_pool(name="qk", bufs=2))
            vbp = p2.enter_context(tc.tile_pool(name="vb", bufs=2))
            ep = p2.enter_context(tc.tile_pool(name="ep", bufs=3))
            trp = p2.enter_context(tc.tile_pool(name="tr", bufs=6))
            rc = p2.enter_context(tc.tile_pool(name="rc", bufs=2))
            ao = p2.enter_context(tc.tile_pool(name="ao", bufs=3))
            rhp = p2.enter_context(tc.tile_pool(name="rh", bufs=1))
            wop = p2.enter_context(tc.tile_pool(name="wo", bufs=3))
            oac = p2.enter_context(tc.tile_pool(name="oac", bufs=1))
            osb = p2.enter_context(tc.tile_pool(name="osb", bufs=4))
            ps_s = p2.enter_context(tc.tile_pool(name="ps_s", bufs=2, space="PSUM"))
            ps_pv = p2.enter_context(tc.tile_pool(name="ps_pv", bufs=2, space="PSUM"))
            ps_o = p2.enter_context(tc.tile_pool(name="ps_o", bufs=2, space="PSUM"))

            for b in range(B):
                vb = vbp.tile([128, LT, OC], BF16, name="vb")
                nc.sync.dma_start(vb[:, :, :],
                                  vsp[b].ap().transpose([1, 0, 2]))
                for h in range(NH):
                    q_sb = qk.tile([128, L], BF16, name="q")
                    nc.sync.dma_start(q_sb[:], qsp[b][h, :, :])
                    k_sb = qk.tile([128, L], BF16, name="k")
                    nc.sync.dma_start(k_sb[:], ksp[b][h, :, :])
                    for half in range(2):
                        q0 = half * 1024
                        pvs = [ps_pv.tile([128, 512], F32, name="ps_pv")
                               for _ in range(2)]
                        tree = []          # bf16 pairwise row-sum tree
                        for kt in range(LT):
                            s_ps = ps_s.tile([128, 1024], F32, name="ps_s")
                            nc.tensor.matmul(s_ps[:, 0:512],
                                             k_sb[:, kt * 128:(kt + 1) * 128],
                                             q_sb[:, q0:q0 + 512],
                                             start=True, stop=True)
                            nc.tensor.matmul(s_ps[:, 512:1024],
                                             k_sb[:, kt * 128:(kt + 1) * 128],
                                             q_sb[:, q0 + 512:q0 + 1024],
                                             start=True, stop=True)
                            e_t = ep.tile([128, 1024], BF16, name="ep")
                            nc.scalar.activation(e_t[:], s_ps[:], EXP_F, scale=SCALE)
                            first, last = (kt == 0), (kt == LT - 1)
                            for c in range(2):
                                nc.tensor.matmul(pvs[c][:],
                                                 vb[:, kt, h * 128:(h + 1) * 128],
                                                 e_t[:, c * 512:(c + 1) * 512],
                                                 start=first, stop=last)
                            node = (0, e_t)
                            while tree and tree[-1][0] == node[0]:
                                prev = tree.pop()
                                nt = trp.tile([128, 1024], BF16, name="tr")
                                nc.vector.tensor_add(nt[:], prev[1][:], node[1][:])
                                node = (node[0] + 1, nt)
                            tree.append(node)
                        assert len(tree) == 1
                        root = tree[0][1]
                        # reduce the 128 partition partials in one f32 matmul
                        r_ps = ps_s.tile([128, 1024], F32, name="ps_s")
                        for c in range(2):
                            nc.tensor.matmul(r_ps[:, c * 512:(c + 1) * 512],
                                             ones_sb[:],
                                             root[:, c * 512:(c + 1) * 512],
                                             start=True, stop=True)
                        rec = rc.tile([128, 1024], F32, name="rc")
                        nc.vector.reciprocal_approx_fast(out=rec[:], in_=r_ps[:])
                        for c in range(2):
                            at = ao.tile([128, 512], BF16, name="ao")
                            nc.vector.tensor_mul(at[:], pvs[c][:],
                                                 rec[:, c * 512:(c + 1) * 512])
                            ci = half * 2 + c
                            nc.gpsimd.dma_start(
                                a2a_in[b][h][2 * ci, :, :], at[:, 0:SHB])
                            nc.gpsimd.dma_start(
                                a2a_in[b][h][2 * ci + 1, :, :], at[:, SHB:2 * SHB])
                    nc.gpsimd.collective_compute(
                        "AllToAll", mybir.AluOpType.bypass,
                        replica_groups=[list(range(NCORES))],
                        ins=[a2a_in[b][h].ap().opt()],
                        outs=[a2a_out[b][h].ap().opt()],
                    )

            # ---- o_proj: two passes over head-groups, SBUF f32 accumulation
            rh = rhp.tile([128, KT, SH], BF16, name="rh")
            rh4 = rh[:].rearrange("p (j f) t -> p j f t", f=4)   # [128,8,4,SH]
            for b in range(B):
                for h in range(NH):
                    # global kt = 4*j + h
                    nc.sync.dma_start(
                        rh4[:, :, h, b * SHB:(b + 1) * SHB],
                        a2a_out[b][h].ap().transpose([1, 0, 2]))
            out_acc = oac.tile([128, D // 128, SH], F32, name="oac")
            # wo dram [ot, p, (j f), o] with f=4; halves f 0:2 / 2:4 are contiguous
            wo4 = wo.ap().rearrange("ot p (j f) o -> ot p j (f o)", f=4)
            for pss in range(2):
                for ot in range(D // 128):
                    wot = wop.tile([128, NCORES, 256], BF16, name="wo")
                    nc.sync.dma_start(
                        wot[:],
                        wo4[ot, :, :, pss * 256:(pss + 1) * 256])
                    po = ps_o.tile([128, SH], F32, name="ps_o")
                    i = 0
                    for hh in (2 * pss, 2 * pss + 1):
                        for j in range(NCORES):
                            nc.tensor.matmul(po[:], wot[:, j, (hh % 2) * 128:(hh % 2) * 128 + 128],
                                             rh[:, 4 * j + hh, :],
                                             start=(i == 0), stop=(i == 15))
                            i += 1
                    if pss == 0:
                        nc.scalar.copy(out_acc[:, ot, :], po[:])
                    else:
                        o_sb = osb.tile([128, SH], F32, name="osb")
                        nc.vector.tensor_add(o_sb[:], po[:], out_acc[:, ot, :])
                        nc.gpsimd.dma_start(out[ot * 128:(ot + 1) * 128, :], o_sb[:])

    nc.compile()
    return nc


def _qk_row_perm():
    # local row order: [h0re|h1re],[h0im|h1im],[h2re|h3re],[h2im|h3im]
    rows = []
    for pr in range(NH // 2):
        ha, hb = 2 * pr, 2 * pr + 1
        rows += [ha * HD + 2 * i for i in range(HD // 2)]
        rows += [hb * HD + 2 * i for i in range(HD // 2)]
        rows += [ha * HD + 2 * i + 1 for i in range(HD // 2)]
        rows += [hb * HD + 2 * i + 1 for i in range(HD // 2)]
    return np.array(rows)


def _prep_inputs(x, freqs_cos, freqs_sin, Wq, Wk, Wv, Wo):
    x = np.asarray(x, np.float32).reshape(T, D)
    Wq, Wk, Wv, Wo = (np.asarray(w, np.float32) for w in (Wq, Wk, Wv, Wo))
    fc = np.asarray(freqs_cos, np.float32)
    fs = np.asarray(freqs_sin, np.float32)

    # shared tensors
    xT = np.ascontiguousarray(
        x.reshape(T, KT, 128).transpose(1, 2, 0)).astype(NPBF16)        # [KT,128,T]
    woh = np.ascontiguousarray(
        Wo.reshape(D // 128, 128, KT, 128).transpose(0, 3, 2, 1)).astype(NPBF16)
    csh = np.ascontiguousarray(np.concatenate([fc.T, fc.T], 0))          # [128,L]
    snh = np.ascontiguousarray(np.concatenate([fs.T, fs.T], 0))
    ones = np.ones([128, 128], NPBF16)

    perm = _qk_row_perm()
    in_maps = []
    for i in range(NCORES):
        rows = slice(OC * i, OC * (i + 1))
        wqi = Wq[rows][perm]                                             # [512, D]
        wki = Wk[rows][perm]
        wqh = np.ascontiguousarray(
            wqi.reshape(NH, 128, KT, 128).transpose(3, 0, 2, 1)).astype(NPBF16)
        wkh = np.ascontiguousarray(
            wki.reshape(NH, 128, KT, 128).transpose(3, 0, 2, 1)).astype(NPBF16)
        wvh = np.ascontiguousarray(
            Wv[rows].reshape(OC, KT, 128).transpose(2, 1, 0)).astype(NPBF16)
        in_maps.append({
            "xT": xT, "wq": wqh, "wk": wkh, "wv": wvh, "wo": woh,
            "cs": csh, "sn": snh, "ones": ones,
        })
    return in_maps


_NC_CACHE = None


def _get_nc():
    global _NC_CACHE
    if _NC_CACHE is None:
        _NC_CACHE = build_nc()
    return _NC_CACHE


def _run(in_maps, trace=False):
    nc = _get_nc()
    res = bass_utils.run_bass_kernel_spmd(
        nc, in_maps, core_ids=list(range(NCORES)), trace=trace)
    return res


def _assemble(results):
    out = np.empty((B, L, D), np.float32)
    for i in range(NCORES):
        o = results[i]["out"]                       # [D, SH] f32
        for b in range(B):
            out[b, SHB * i:SHB * (i + 1), :] = o[:, b * SHB:(b + 1) * SHB].T
    return out


def kernel(x, freqs_cos, freqs_sin, Wq, Wk, Wv, Wo):
    in_maps = _prep_inputs(x, freqs_cos, freqs_sin, Wq, Wk, Wv, Wo)
    res = _run(in_maps, trace=False)
    return _assemble(res.results)


# revision 14
# speedup vs baseline: 1.1510x; 1.0216x over previous
"""Distributed multi-head attention (B=2, L=2048, D=4096, H=32) on 8 TRN2 NeuronCores.

Strategy: tensor-parallel over heads (4 heads/core) for QKV+attention, then an
AllToAll that trades head-dims for token-slices so o_proj is token-sharded
(each core computes out[:, its 512 tokens] with the full Wo) — the AllToAll
moves 4 MB/core instead of the 64 MB/core an output AllReduce would.

All matmuls run in bf16 on the TensorEngine (f32 PSUM accumulation).
Host-side prep: transpose/permute/tile weights and x into DMA-friendly
partition-major layouts, pre-cast to bf16. Host post: concatenate the 8
token-shards and transpose. Verified rel-err vs f32 reference ~6e-3.

RoPE trick: Q/K output columns are permuted host-side (per head: even dims
then odd dims, pairs of heads interleaved into 128-row tiles) so the rotation
becomes full-width [128, t] vector ops with no partition-pair shuffles.

Perf notes: all multi-tile loads are single dma_starts; stores go on the
GpSimd DMA queue, loads on Sync.  The softmax row-sum is a bf16 pairwise
tree on the VectorEngine (533ns/add in 2x mode; the depth-4 bf16 error
averages out in the final f32 ones-matmul partition-reduce, measured
<1e-3 on the row sums) so the TensorEngine only runs scores+PV in the
attention inner loop.  The AllToAll is split 8 ways (per batch x head) and
issued as each head finishes, so only the last ~0.5MB exchange is exposed.
o_proj runs as two head-group passes accumulating in SBUF f32 — the first
pass's matmuls fill TensorEngine bubbles during the ACT-bound attention
phase, the second pass needs only the last exchanges.  Measured
1.54ms on silicon (PE-array busy ~86%, QKV phase >99%).
"""

import sys

if "/opt/trn_rl_repo" not in sys.path:
    sys.path.insert(0, "/opt/trn_rl_repo")

from contextlib import ExitStack

import ml_dtypes
import numpy as np

import concourse.bass as bass
import concourse.tile as tile
from concourse import bacc, mybir
from concourse import bass_utils

BF16 = mybir.dt.bfloat16
F32 = mybir.dt.float32
NPBF16 = ml_dtypes.bfloat16

NCORES = 8
B, L, D, H, HD = 2, 2048, 4096, 32, 128
T = B * L              # 4096 global tokens
NH = H // NCORES       # 4 heads per core
OC = NH * HD           # 512 projection dims per core
KT = D // 128          # 32 contraction tiles over D
LT = L // 128          # 16 key tiles per batch
TG = 512               # phase-1 token-group width
NG = T // TG           # 8 groups
SH = T // NCORES       # 512 output tokens per core
SHB = SH // B          # 256 per batch
SCALE = 1.0 / float(np.sqrt(HD))

EXP_F = mybir.ActivationFunctionType.Exp


def build_nc():
    nc = bacc.Bacc("TRN2", target_bir_lowering=False, debug=False,
                   num_devices=NCORES)

    # ---- I/O (per-core shards, host-pretiled, bf16) ----
    xT = nc.dram_tensor("xT", [KT, 128, T], BF16, kind="ExternalInput")
    wq = nc.dram_tensor("wq", [128, NH, KT, 128], BF16, kind="ExternalInput")
    wk = nc.dram_tensor("wk", [128, NH, KT, 128], BF16, kind="ExternalInput")
    wv = nc.dram_tensor("wv", [128, KT, OC], BF16, kind="ExternalInput")
    wo = nc.dram_tensor("wo", [D // 128, 128, KT, 128], BF16, kind="ExternalInput")
    cs = nc.dram_tensor("cs", [128, L], F32, kind="ExternalInput")
    sn = nc.dram_tensor("sn", [128, L], F32, kind="ExternalInput")
    ones = nc.dram_tensor("ones", [128, 128], BF16, kind="ExternalInput")
    out = nc.dram_tensor("out", [D, SH], F32, kind="ExternalOutput")

    # ---- internal DRAM (spills + collective bounce) ----
    qsp = [nc.dram_tensor(f"qsp{b}", [NH, 128, L], BF16) for b in range(B)]
    ksp = [nc.dram_tensor(f"ksp{b}", [NH, 128, L], BF16) for b in range(B)]
    vsp = [nc.dram_tensor(f"vsp{b}", [LT, 128, OC], BF16) for b in range(B)]
    # AllToAll split by (batch, head): [shard, HD dims, SHB tokens]
    a2a_in = [[nc.dram_tensor(f"a2ai{b}_{h}", [NCORES, HD, SHB], BF16)
               for h in range(NH)] for b in range(B)]
    a2a_out = [[nc.dram_tensor(f"a2ao{b}_{h}", [NCORES, HD, SHB], BF16)
                for h in range(NH)] for b in range(B)]

    with tile.TileContext(nc) as tc, ExitStack() as ctx:
        singles = ctx.enter_context(tc.tile_pool(name="singles", bufs=1))
        ones_sb = singles.tile([128, 128], BF16, name="ones")
        nc.sync.dma_start(ones_sb[:], ones[:, :])

        # ================= Phase 1: QKV projections + RoPE =================
        with ExitStack() as p1:
            wpool = p1.enter_context(tc.tile_pool(name="w", bufs=1))
            wq_sb = wpool.tile([128, NH, KT, 128], BF16, name="wq")
            nc.sync.dma_start(wq_sb[:], wq[:, :, :, :])
            wk_sb = wpool.tile([128, NH, KT, 128], BF16, name="wk")
            nc.sync.dma_start(wk_sb[:], wk[:, :, :, :])
            wv_sb = wpool.tile([128, KT, OC], BF16, name="wv")
            nc.sync.dma_start(wv_sb[:], wv[:, :, :])

            xpool = p1.enter_context(tc.tile_pool(name="xg", bufs=2))
            cpool = p1.enter_context(tc.tile_pool(name="csg", bufs=2))
            tmp = p1.enter_context(tc.tile_pool(name="tmp", bufs=8))
            st = p1.enter_context(tc.tile_pool(name="st", bufs=6))
            ps1 = p1.enter_context(tc.tile_pool(name="ps1", bufs=6, space="PSUM"))

            for g in range(NG):
                b = g // (NG // B)
                pos0 = (g % (NG // B)) * TG          # position within batch
                xg = xpool.tile([128, KT, TG], BF16, name="xg")
                nc.sync.dma_start(
                    xg[:, :, :],
                    xT[:, :, g * TG:(g + 1) * TG].transpose([1, 0, 2]))
                csg = cpool.tile([128, TG], F32, name="csg")
                nc.sync.dma_start(csg[:], cs[:, pos0:pos0 + TG])
                sng = cpool.tile([128, TG], F32, name="sng")
                nc.sync.dma_start(sng[:], sn[:, pos0:pos0 + TG])

                # Q and K with fused RoPE
                for wsb, sp in ((wq_sb, qsp[b]), (wk_sb, ksp[b])):
                    for pr in range(NH // 2):
                        p_re = ps1.tile([128, TG], F32, name="ps1")
                        p_im = ps1.tile([128, TG], F32, name="ps1")
                        for kt in range(KT):
                            nc.tensor.matmul(p_re[:], wsb[:, 2 * pr, kt, :],
                                             xg[:, kt, :],
                                             start=(kt == 0), stop=(kt == KT - 1))
                        for kt in range(KT):
                            nc.tensor.matmul(p_im[:], wsb[:, 2 * pr + 1, kt, :],
                                             xg[:, kt, :],
                                             start=(kt == 0), stop=(kt == KT - 1))
                        t1 = tmp.tile([128, TG], F32, name="tmp")
                        t2 = tmp.tile([128, TG], F32, name="tmp")
                        t3 = tmp.tile([128, TG], F32, name="tmp")
                        t4 = tmp.tile([128, TG], F32, name="tmp")
                        o_re = st.tile([128, TG], BF16, name="st")
                        o_im = st.tile([128, TG], BF16, name="st")
                        nc.vector.tensor_mul(t1[:], p_re[:], csg[:])
                        nc.vector.tensor_mul(t2[:], p_im[:], sng[:])
                        nc.vector.tensor_sub(o_re[:], t1[:], t2[:])
                        nc.vector.tensor_mul(t3[:], p_re[:], sng[:])
                        nc.vector.tensor_mul(t4[:], p_im[:], csg[:])
                        nc.vector.tensor_add(o_im[:], t3[:], t4[:])
                        ha, hb = 2 * pr, 2 * pr + 1
                        nc.gpsimd.dma_start(sp[ha, 0:64, pos0:pos0 + TG],
                                            o_re[0:64, :])
                        nc.gpsimd.dma_start(sp[hb, 0:64, pos0:pos0 + TG],
                                            o_re[64:128, :])
                        nc.gpsimd.dma_start(sp[ha, 64:128, pos0:pos0 + TG],
                                            o_im[0:64, :])
                        nc.gpsimd.dma_start(sp[hb, 64:128, pos0:pos0 + TG],
                                            o_im[64:128, :])

                # V (layout [t, oc])
                for sub in range(TG // 128):
                    pv = ps1.tile([128, OC], F32, name="ps1")
                    for kt in range(KT):
                        nc.tensor.matmul(pv[:], xg[:, kt, sub * 128:(sub + 1) * 128],
                                         wv_sb[:, kt, :],
                                         start=(kt == 0), stop=(kt == KT - 1))
                    vo = st.tile([128, OC], BF16, name="st")
                    nc.scalar.copy(vo[:], pv[:])
                    tt = pos0 // 128 + sub
                    nc.gpsimd.dma_start(vsp[b][tt, :, :], vo[:])

        # ============ Phase 2+3: attention, AllToAll, o_proj (overlapped) =====
        with ExitStack() as p2:
            qk = p2.enter_context(tc.tile_pool(name="qk", bufs=2))
            vbp = p2.enter_context(tc.tile_pool(name="vb", bufs=2))
            ep = p2.enter_context(tc.tile_pool(name="ep", bufs=4))
            pvc = p2.enter_context(tc.tile_pool(name="pvc", bufs=4))
            trp = p2.enter_context(tc.tile_pool(name="tr", bufs=6))
            rc = p2.enter_context(tc.tile_pool(name="rc", bufs=4))
            ao = p2.enter_context(tc.tile_pool(name="ao", bufs=3))
            rhp = p2.enter_context(tc.tile_pool(name="rh", bufs=1))
            wop = p2.enter_context(tc.tile_pool(name="wo", bufs=3))
            oac = p2.enter_context(tc.tile_pool(name="oac", bufs=1))
            osb = p2.enter_context(tc.tile_pool(name="osb", bufs=4))
            ps_s = p2.enter_context(tc.tile_pool(name="ps_s", bufs=2, space="PSUM"))
            ps_pv = p2.enter_context(tc.tile_pool(name="ps_pv", bufs=2, space="PSUM"))
            ps_o = p2.enter_context(tc.tile_pool(name="ps_o", bufs=2, space="PSUM"))

            for b in range(B):
                vb = vbp.tile([128, LT, OC], BF16, name="vb")
                nc.sync.dma_start(vb[:, :, :],
                                  vsp[b].ap().transpose([1, 0, 2]))
                for h in range(NH):
                    q_sb = qk.tile([128, L], BF16, name="q")
                    nc.sync.dma_start(q_sb[:], qsp[b][h, :, :])
                    k_sb = qk.tile([128, L], BF16, name="k")
                    nc.sync.dma_start(k_sb[:], ksp[b][h, :, :])
                    for half in range(2):
                        q0 = half * 1024
                        pvs = [ps_pv.tile([128, 512], F32, name="ps_pv")
                               for _ in range(2)]
                        tree = []          # bf16 pairwise row-sum tree
                        for kt in range(LT):
                            s_ps = ps_s.tile([128, 1024], F32, name="ps_s")
                            nc.tensor.matmul(s_ps[:, 0:512],
                                             k_sb[:, kt * 128:(kt + 1) * 128],
                                             q_sb[:, q0:q0 + 512],
                                             start=True, stop=True)
                            nc.tensor.matmul(s_ps[:, 512:1024],
                                             k_sb[:, kt * 128:(kt + 1) * 128],
                                             q_sb[:, q0 + 512:q0 + 1024],
                                             start=True, stop=True)
                            e_t = ep.tile([128, 1024], BF16, name="ep")
                            nc.scalar.activation(e_t[:], s_ps[:], EXP_F, scale=SCALE)
                            first, last = (kt == 0), (kt == LT - 1)
                            for c in range(2):
                                nc.tensor.matmul(pvs[c][:],
                                                 vb[:, kt, h * 128:(h + 1) * 128],
                                                 e_t[:, c * 512:(c + 1) * 512],
                                                 start=first, stop=last)
                            node = (0, e_t)
                            while tree and tree[-1][0] == node[0]:
                                prev = tree.pop()
                                nt = trp.tile([128, 1024], BF16, name="tr")
                                nc.vector.tensor_add(nt[:], prev[1][:], node[1][:])
                                node = (node[0] + 1, nt)
                            tree.append(node)
                        assert len(tree) == 1
                        root = tree[0][1]
                        # drain pv psums to SBUF so next half's MMs start now
                        pvcs = []
                        for c in range(2):
                            pc = pvc.tile([128, 512], F32, name="pvc")
                            nc.vector.tensor_copy(pc[:], pvs[c][:])
                            pvcs.append(pc)
                        # partition-reduce the row-sum tree root (pv slots free)
                        rts = [ps_pv.tile([128, 512], F32, name="ps_pv")
                               for _ in range(2)]
                        for c in range(2):
                            nc.tensor.matmul(rts[c][:], ones_sb[:],
                                             root[:, c * 512:(c + 1) * 512],
                                             start=True, stop=True)
                        for c in range(2):
                            rec = rc.tile([128, 512], F32, name="rc")
                            nc.vector.reciprocal_approx_fast(out=rec[:],
                                                             in_=rts[c][:])
                            at = ao.tile([128, 512], BF16, name="ao")
                            nc.vector.tensor_mul(at[:], pvcs[c][:], rec[:])
                            ci = half * 2 + c
                            nc.gpsimd.dma_start(
                                a2a_in[b][h][2 * ci, :, :], at[:, 0:SHB])
                            nc.gpsimd.dma_start(
                                a2a_in[b][h][2 * ci + 1, :, :], at[:, SHB:2 * SHB])
                    nc.gpsimd.collective_compute(
                        "AllToAll", mybir.AluOpType.bypass,
                        replica_groups=[list(range(NCORES))],
                        ins=[a2a_in[b][h].ap().opt()],
                        outs=[a2a_out[b][h].ap().opt()],
                    )

            # ---- o_proj: two passes over head-groups, SBUF f32 accumulation
            rh = rhp.tile([128, KT, SH], BF16, name="rh")
            rh4 = rh[:].rearrange("p (j f) t -> p j f t", f=4)   # [128,8,4,SH]
            for b in range(B):
                for h in range(NH):
                    # global kt = 4*j + h
                    nc.sync.dma_start(
                        rh4[:, :, h, b * SHB:(b + 1) * SHB],
                        a2a_out[b][h].ap().transpose([1, 0, 2]))
            out_acc = oac.tile([128, D // 128, SH], F32, name="oac")
            # wo dram [ot, p, (j f), o] with f=4; halves f 0:2 / 2:4 are contiguous
            wo4 = wo.ap().rearrange("ot p (j f) o -> ot p j (f o)", f=4)
            for pss in range(2):
                for ot in range(D // 128):
                    wot = wop.tile([128, NCORES, 256], BF16, name="wo")
                    nc.sync.dma_start(
                        wot[:],
                        wo4[ot, :, :, pss * 256:(pss + 1) * 256])
                    po = ps_o.tile([128, SH], F32, name="ps_o")
                    i = 0
                    for hh in (2 * pss, 2 * pss + 1):
                        for j in range(NCORES):
                            nc.tensor.matmul(po[:], wot[:, j, (hh % 2) * 128:(hh % 2) * 128 + 128],
                                             rh[:, 4 * j + hh, :],
                                             start=(i == 0), stop=(i == 15))
                            i += 1
                    if pss == 0:
                        nc.scalar.copy(out_acc[:, ot, :], po[:])
                    else:
                        o_sb = osb.tile([128, SH], F32, name="osb")
                        nc.vector.tensor_add(o_sb[:], po[:], out_acc[:, ot, :])
                        nc.gpsimd.dma_start(out[ot * 128:(ot + 1) * 128, :], o_sb[:])

    nc.compile()
    return nc


def _qk_row_perm():
    # local row order: [h0re|h1re],[h0im|h1im],[h2re|h3re],[h2im|h3im]
    rows = []
    for pr in range(NH // 2):
        ha, hb = 2 * pr, 2 * pr + 1
        rows += [ha * HD + 2 * i for i in range(HD // 2)]
        rows += [hb * HD + 2 * i for i in range(HD // 2)]
        rows += [ha * HD + 2 * i + 1 for i in range(HD // 2)]
        rows += [hb * HD + 2 * i + 1 for i in range(HD // 2)]
    return np.array(rows)


def _prep_inputs(x, freqs_cos, freqs_sin, Wq, Wk, Wv, Wo):
    x = np.asarray(x, np.float32).reshape(T, D)
    Wq, Wk, Wv, Wo = (np.asarray(w, np.float32) for w in (Wq, Wk, Wv, Wo))
    fc = np.asarray(freqs_cos, np.float32)
    fs = np.asarray(freqs_sin, np.float32)

    # shared tensors
    xT = np.ascontiguousarray(
        x.reshape(T, KT, 128).transpose(1, 2, 0)).astype(NPBF16)        # [KT,128,T]
    woh = np.ascontiguousarray(
        Wo.reshape(D // 128, 128, KT, 128).transpose(0, 3, 2, 1)).astype(NPBF16)
    csh = np.ascontiguousarray(np.concatenate([fc.T, fc.T], 0))          # [128,L]
    snh = np.ascontiguousarray(np.concatenate([fs.T, fs.T], 0))
    ones = np.ones([128, 128], NPBF16)

    perm = _qk_row_perm()
    in_maps = []
    for i in range(NCORES):
        rows = slice(OC * i, OC * (i + 1))
        wqi = Wq[rows][perm]                                             # [512, D]
        wki = Wk[rows][perm]
        wqh = np.ascontiguousarray(
            wqi.reshape(NH, 128, KT, 128).transpose(3, 0, 2, 1)).astype(NPBF16)
        wkh = np.ascontiguousarray(
            wki.reshape(NH, 128, KT, 128).transpose(3, 0, 2, 1)).astype(NPBF16)
        wvh = np.ascontiguousarray(
            Wv[rows].reshape(OC, KT, 128).transpose(2, 1, 0)).astype(NPBF16)
        in_maps.append({
            "xT": xT, "wq": wqh, "wk": wkh, "wv": wvh, "wo": woh,
            "cs": csh, "sn": snh, "ones": ones,
        })
    return in_maps


_NC_CACHE = None


def _get_nc():
    global _NC_CACHE
    if _NC_CACHE is None:
        _NC_CACHE = build_nc()
    return _NC_CACHE


def _run(in_maps, trace=False):
    nc = _get_nc()
    res = bass_utils.run_bass_kernel_spmd(
        nc, in_maps, core_ids=list(range(NCORES)), trace=trace)
    return res


def _assemble(results):
    out = np.empty((B, L, D), np.float32)
    for i in range(NCORES):
        o = results[i]["out"]                       # [D, SH] f32
        for b in range(B):
            out[b, SHB * i:SHB * (i + 1), :] = o[:, b * SHB:(b + 1) * SHB].T
    return out


def kernel(x, freqs_cos, freqs_sin, Wq, Wk, Wv, Wo):
    in_maps = _prep_inputs(x, freqs_cos, freqs_sin, Wq, Wk, Wv, Wo)
    res = _run(in_maps, trace=False)
    return _assemble(res.results)


# revision 16
# speedup vs baseline: 1.1620x; 1.0095x over previous
"""Distributed multi-head attention (B=2, L=2048, D=4096, H=32) on 8 TRN2 NeuronCores.

Strategy: tensor-parallel over heads (4 heads/core) for QKV+attention, then an
AllToAll that trades head-dims for token-slices so o_proj is token-sharded
(each core computes out[:, its 512 tokens] with the full Wo) — the AllToAll
moves 4 MB/core instead of the 64 MB/core an output AllReduce would.

All matmuls run in bf16 on the TensorEngine (f32 PSUM accumulation).
Host-side prep: transpose/permute/tile weights and x into DMA-friendly
partition-major layouts, pre-cast to bf16. Host post: concatenate the 8
token-shards and transpose. Verified rel-err vs f32 reference ~6e-3.

RoPE trick: Q/K output columns are permuted host-side (per head: even dims
then odd dims, pairs of heads interleaved into 128-row tiles) so the rotation
becomes full-width [128, t] vector ops with no partition-pair shuffles.

Perf notes: all multi-tile loads are single dma_starts; stores go on the
GpSimd DMA queue, loads on Sync.  The softmax row-sum is a bf16 pairwise
tree on the VectorEngine (533ns/add in 2x mode; the depth-4 bf16 error
averages out in the final f32 ones-matmul partition-reduce, measured
<1e-3 on the row sums) so the TensorEngine only runs scores+PV in the
attention inner loop.  The AllToAll is split 8 ways (per batch x head) and
issued as each head finishes, so only the last ~0.5MB exchange is exposed.
o_proj runs as two head-group passes accumulating in SBUF f32 — the first
pass's matmuls fill TensorEngine bubbles during the ACT-bound attention
phase, the second pass needs only the last exchanges.  Measured
1.54ms on silicon (PE-array busy ~86%, QKV phase >99%).
"""

import sys

if "/opt/trn_rl_repo" not in sys.path:
    sys.path.insert(0, "/opt/trn_rl_repo")

from contextlib import ExitStack

import ml_dtypes
import numpy as np

import concourse.bass as bass
import concourse.tile as tile
from concourse import bacc, mybir
from concourse import bass_utils

BF16 = mybir.dt.bfloat16
F32 = mybir.dt.float32
NPBF16 = ml_dtypes.bfloat16

NCORES = 8
B, L, D, H, HD = 2, 2048, 4096, 32, 128
T = B * L              # 4096 global tokens
NH = H // NCORES       # 4 heads per core
OC = NH * HD           # 512 projection dims per core
KT = D // 128          # 32 contraction tiles over D
LT = L // 128          # 16 key tiles per batch
TG = 512               # phase-1 token-group width
NG = T // TG           # 8 groups
SH = T // NCORES       # 512 output tokens per core
SHB = SH // B          # 256 per batch
SCALE = 1.0 / float(np.sqrt(HD))

EXP_F = mybir.ActivationFunctionType.Exp


def build_nc():
    nc = bacc.Bacc("TRN2", target_bir_lowering=False, debug=False,
                   num_devices=NCORES)

    # ---- I/O (per-core shards, host-pretiled, bf16) ----
    xT = nc.dram_tensor("xT", [KT, 128, T], BF16, kind="ExternalInput")
    wq = nc.dram_tensor("wq", [128, NH, KT, 128], BF16, kind="ExternalInput")
    wk = nc.dram_tensor("wk", [128, NH, KT, 128], BF16, kind="ExternalInput")
    wv = nc.dram_tensor("wv", [128, KT, OC], BF16, kind="ExternalInput")
    wo = nc.dram_tensor("wo", [D // 128, 128, KT, 128], BF16, kind="ExternalInput")
    cs = nc.dram_tensor("cs", [128, L], F32, kind="ExternalInput")
    sn = nc.dram_tensor("sn", [128, L], F32, kind="ExternalInput")
    ones = nc.dram_tensor("ones", [128, 128], BF16, kind="ExternalInput")
    out = nc.dram_tensor("out", [D, SH], F32, kind="ExternalOutput")

    # ---- internal DRAM (spills + collective bounce) ----
    qsp = [nc.dram_tensor(f"qsp{b}", [NH, 128, L], BF16) for b in range(B)]
    ksp = [nc.dram_tensor(f"ksp{b}", [NH, 128, L], BF16) for b in range(B)]
    vsp = [nc.dram_tensor(f"vsp{b}", [LT, 128, OC], BF16) for b in range(B)]
    # AllToAll split by (batch, head): [shard, HD dims, SHB tokens]
    a2a_in = [[nc.dram_tensor(f"a2ai{b}_{h}", [NCORES, HD, SHB], BF16)
               for h in range(NH)] for b in range(B)]
    a2a_out = [[nc.dram_tensor(f"a2ao{b}_{h}", [NCORES, HD, SHB], BF16)
                for h in range(NH)] for b in range(B)]

    with tile.TileContext(nc) as tc, ExitStack() as ctx:
        singles = ctx.enter_context(tc.tile_pool(name="singles", bufs=1))
        ones_sb = singles.tile([128, 128], BF16, name="ones")
        nc.sync.dma_start(ones_sb[:], ones[:, :])

        # ================= Phase 1: QKV projections + RoPE =================
        with ExitStack() as p1:
            wpool = p1.enter_context(tc.tile_pool(name="w", bufs=1))
            wq_sb = wpool.tile([128, NH, KT, 128], BF16, name="wq")
            nc.sync.dma_start(wq_sb[:], wq[:, :, :, :])
            wk_sb = wpool.tile([128, NH, KT, 128], BF16, name="wk")
            nc.sync.dma_start(wk_sb[:], wk[:, :, :, :])
            wv_sb = wpool.tile([128, KT, OC], BF16, name="wv")
            nc.sync.dma_start(wv_sb[:], wv[:, :, :])

            xpool = p1.enter_context(tc.tile_pool(name="xg", bufs=2))
            cpool = p1.enter_context(tc.tile_pool(name="csg", bufs=2))
            tmp = p1.enter_context(tc.tile_pool(name="tmp", bufs=8))
            st = p1.enter_context(tc.tile_pool(name="st", bufs=6))
            ps1 = p1.enter_context(tc.tile_pool(name="ps1", bufs=6, space="PSUM"))

            for g in range(NG):
                b = g // (NG // B)
                pos0 = (g % (NG // B)) * TG          # position within batch
                xg = xpool.tile([128, KT, TG], BF16, name="xg")
                nc.sync.dma_start(
                    xg[:, :, :],
                    xT[:, :, g * TG:(g + 1) * TG].transpose([1, 0, 2]))
                csg = cpool.tile([128, TG], F32, name="csg")
                nc.sync.dma_start(csg[:], cs[:, pos0:pos0 + TG])
                sng = cpool.tile([128, TG], F32, name="sng")
                nc.sync.dma_start(sng[:], sn[:, pos0:pos0 + TG])

                # Q and K with fused RoPE
                for wsb, sp in ((wq_sb, qsp[b]), (wk_sb, ksp[b])):
                    for pr in range(NH // 2):
                        p_re = ps1.tile([128, TG], F32, name="ps1")
                        p_im = ps1.tile([128, TG], F32, name="ps1")
                        for kt in range(KT):
                            nc.tensor.matmul(p_re[:], wsb[:, 2 * pr, kt, :],
                                             xg[:, kt, :],
                                             start=(kt == 0), stop=(kt == KT - 1))
                        for kt in range(KT):
                            nc.tensor.matmul(p_im[:], wsb[:, 2 * pr + 1, kt, :],
                                             xg[:, kt, :],
                                             start=(kt == 0), stop=(kt == KT - 1))
                        t1 = tmp.tile([128, TG], F32, name="tmp")
                        t2 = tmp.tile([128, TG], F32, name="tmp")
                        t3 = tmp.tile([128, TG], F32, name="tmp")
                        t4 = tmp.tile([128, TG], F32, name="tmp")
                        o_re = st.tile([128, TG], BF16, name="st")
                        o_im = st.tile([128, TG], BF16, name="st")
                        nc.vector.tensor_mul(t1[:], p_re[:], csg[:])
                        nc.vector.tensor_mul(t2[:], p_im[:], sng[:])
                        nc.vector.tensor_sub(o_re[:], t1[:], t2[:])
                        nc.vector.tensor_mul(t3[:], p_re[:], sng[:])
                        nc.vector.tensor_mul(t4[:], p_im[:], csg[:])
                        nc.vector.tensor_add(o_im[:], t3[:], t4[:])
                        ha, hb = 2 * pr, 2 * pr + 1
                        nc.gpsimd.dma_start(sp[ha, 0:64, pos0:pos0 + TG],
                                            o_re[0:64, :])
                        nc.gpsimd.dma_start(sp[hb, 0:64, pos0:pos0 + TG],
                                            o_re[64:128, :])
                        nc.gpsimd.dma_start(sp[ha, 64:128, pos0:pos0 + TG],
                                            o_im[0:64, :])
                        nc.gpsimd.dma_start(sp[hb, 64:128, pos0:pos0 + TG],
                                            o_im[64:128, :])

                # V (layout [t, oc])
                for sub in range(TG // 128):
                    pv = ps1.tile([128, OC], F32, name="ps1")
                    for kt in range(KT):
                        nc.tensor.matmul(pv[:], xg[:, kt, sub * 128:(sub + 1) * 128],
                                         wv_sb[:, kt, :],
                                         start=(kt == 0), stop=(kt == KT - 1))
                    vo = st.tile([128, OC], BF16, name="st")
                    nc.scalar.copy(vo[:], pv[:])
                    tt = pos0 // 128 + sub
                    nc.gpsimd.dma_start(vsp[b][tt, :, :], vo[:])

        # ============ Phase 2+3: attention, AllToAll, o_proj (overlapped) =====
        with ExitStack() as p2:
            qk = p2.enter_context(tc.tile_pool(name="qk", bufs=2))
            vbp = p2.enter_context(tc.tile_pool(name="vb", bufs=2))
            ep = p2.enter_context(tc.tile_pool(name="ep", bufs=4))
            pvc = p2.enter_context(tc.tile_pool(name="pvc", bufs=4))
            trp = p2.enter_context(tc.tile_pool(name="tr", bufs=6))
            rc = p2.enter_context(tc.tile_pool(name="rc", bufs=4))
            ao = p2.enter_context(tc.tile_pool(name="ao", bufs=3))
            rhp = p2.enter_context(tc.tile_pool(name="rh", bufs=1))
            wop = p2.enter_context(tc.tile_pool(name="wo", bufs=3))
            oac = p2.enter_context(tc.tile_pool(name="oac", bufs=1))
            osb = p2.enter_context(tc.tile_pool(name="osb", bufs=4))
            ps_s = p2.enter_context(tc.tile_pool(name="ps_s", bufs=2, space="PSUM"))
            ps_pv = p2.enter_context(tc.tile_pool(name="ps_pv", bufs=2, space="PSUM"))
            ps_o = p2.enter_context(tc.tile_pool(name="ps_o", bufs=2, space="PSUM"))

            for b in range(B):
                vb = vbp.tile([128, LT, OC], BF16, name="vb")
                nc.sync.dma_start(vb[:, :, :],
                                  vsp[b].ap().transpose([1, 0, 2]))
                for h in range(NH):
                    # scalar-engine DMA queue: decoupled from the phase-1
                    # sync-queue backlog, so these prefetch during QKV
                    q_sb = qk.tile([128, L], BF16, name="q")
                    nc.scalar.dma_start(q_sb[:], qsp[b][h, :, :])
                    k_sb = qk.tile([128, L], BF16, name="k")
                    nc.scalar.dma_start(k_sb[:], ksp[b][h, :, :])
                    for half in range(2):
                        q0 = half * 1024
                        pvs = [ps_pv.tile([128, 512], F32, name="ps_pv")
                               for _ in range(2)]
                        tree = []          # bf16 pairwise row-sum tree
                        for kt in range(LT):
                            s_ps = ps_s.tile([128, 1024], F32, name="ps_s")
                            nc.tensor.matmul(s_ps[:, 0:512],
                                             k_sb[:, kt * 128:(kt + 1) * 128],
                                             q_sb[:, q0:q0 + 512],
                                             start=True, stop=True)
                            nc.tensor.matmul(s_ps[:, 512:1024],
                                             k_sb[:, kt * 128:(kt + 1) * 128],
                                             q_sb[:, q0 + 512:q0 + 1024],
                                             start=True, stop=True)
                            e_t = ep.tile([128, 1024], BF16, name="ep")
                            nc.scalar.activation(e_t[:], s_ps[:], EXP_F, scale=SCALE)
                            first, last = (kt == 0), (kt == LT - 1)
                            for c in range(2):
                                nc.tensor.matmul(pvs[c][:],
                                                 vb[:, kt, h * 128:(h + 1) * 128],
                                                 e_t[:, c * 512:(c + 1) * 512],
                                                 start=first, stop=last)
                            node = (0, e_t)
                            while tree and tree[-1][0] == node[0]:
                                prev = tree.pop()
                                nt = trp.tile([128, 1024], BF16, name="tr")
                                nc.vector.tensor_add(nt[:], prev[1][:], node[1][:])
                                node = (node[0] + 1, nt)
                            tree.append(node)
                        assert len(tree) == 1
                        root = tree[0][1]
                        # drain pv psums to SBUF so next half's MMs start now
                        pvcs = []
                        for c in range(2):
                            pc = pvc.tile([128, 512], F32, name="pvc")
                            nc.vector.tensor_copy(pc[:], pvs[c][:])
                            pvcs.append(pc)
                        # partition-reduce the row-sum tree root (pv slots free)
                        rts = [ps_pv.tile([128, 512], F32, name="ps_pv")
                               for _ in range(2)]
                        for c in range(2):
                            nc.tensor.matmul(rts[c][:], ones_sb[:],
                                             root[:, c * 512:(c + 1) * 512],
                                             start=True, stop=True)
                        for c in range(2):
                            rec = rc.tile([128, 512], F32, name="rc")
                            nc.vector.reciprocal_approx_fast(out=rec[:],
                                                             in_=rts[c][:])
                            at = ao.tile([128, 512], BF16, name="ao")
                            nc.vector.tensor_mul(at[:], pvcs[c][:], rec[:])
                            ci = half * 2 + c
                            nc.gpsimd.dma_start(
                                a2a_in[b][h][2 * ci, :, :], at[:, 0:SHB])
                            nc.gpsimd.dma_start(
                                a2a_in[b][h][2 * ci + 1, :, :], at[:, SHB:2 * SHB])
                    nc.gpsimd.collective_compute(
                        "AllToAll", mybir.AluOpType.bypass,
                        replica_groups=[list(range(NCORES))],
                        ins=[a2a_in[b][h].ap().opt()],
                        outs=[a2a_out[b][h].ap().opt()],
                    )

            # ---- o_proj: two passes over head-groups, SBUF f32 accumulation
            rh = rhp.tile([128, KT, SH], BF16, name="rh")
            rh4 = rh[:].rearrange("p (j f) t -> p j f t", f=4)   # [128,8,4,SH]
            for b in range(B):
                for h in range(NH):
                    # global kt = 4*j + h
                    nc.sync.dma_start(
                        rh4[:, :, h, b * SHB:(b + 1) * SHB],
                        a2a_out[b][h].ap().transpose([1, 0, 2]))
            out_acc = oac.tile([128, D // 128, SH], F32, name="oac")
            # wo dram [ot, p, (j f), o] with f=4; halves f 0:2 / 2:4 are contiguous
            wo4 = wo.ap().rearrange("ot p (j f) o -> ot p j (f o)", f=4)
            for pss in range(2):
                for ot in range(D // 128):
                    wot = wop.tile([128, NCORES, 256], BF16, name="wo")
                    nc.sync.dma_start(
                        wot[:],
                        wo4[ot, :, :, pss * 256:(pss + 1) * 256])
                    po = ps_o.tile([128, SH], F32, name="ps_o")
                    i = 0
                    for hh in (2 * pss, 2 * pss + 1):
                        for j in range(NCORES):
                            nc.tensor.matmul(po[:], wot[:, j, (hh % 2) * 128:(hh % 2) * 128 + 128],
                                             rh[:, 4 * j + hh, :],
                                             start=(i == 0), stop=(i == 15))
                            i += 1
                    if pss == 0:
                        nc.scalar.copy(out_acc[:, ot, :], po[:])
                    else:
                        o_sb = osb.tile([128, SH], F32, name="osb")
                        nc.vector.tensor_add(o_sb[:], po[:], out_acc[:, ot, :])
                        nc.gpsimd.dma_start(out[ot * 128:(ot + 1) * 128, :], o_sb[:])

    nc.compile()
    return nc


def _qk_row_perm():
    # local row order: [h0re|h1re],[h0im|h1im],[h2re|h3re],[h2im|h3im]
    rows = []
    for pr in range(NH // 2):
        ha, hb = 2 * pr, 2 * pr + 1
        rows += [ha * HD + 2 * i for i in range(HD // 2)]
        rows += [hb * HD + 2 * i for i in range(HD // 2)]
        rows += [ha * HD + 2 * i + 1 for i in range(HD // 2)]
        rows += [hb * HD + 2 * i + 1 for i in range(HD // 2)]
    return np.array(rows)


def _prep_inputs(x, freqs_cos, freqs_sin, Wq, Wk, Wv, Wo):
    x = np.asarray(x, np.float32).reshape(T, D)
    Wq, Wk, Wv, Wo = (np.asarray(w, np.float32) for w in (Wq, Wk, Wv, Wo))
    fc = np.asarray(freqs_cos, np.float32)
    fs = np.asarray(freqs_sin, np.float32)

    # shared tensors
    xT = np.ascontiguousarray(
        x.reshape(T, KT, 128).transpose(1, 2, 0)).astype(NPBF16)        # [KT,128,T]
    woh = np.ascontiguousarray(
        Wo.reshape(D // 128, 128, KT, 128).transpose(0, 3, 2, 1)).astype(NPBF16)
    csh = np.ascontiguousarray(np.concatenate([fc.T, fc.T], 0))          # [128,L]
    snh = np.ascontiguousarray(np.concatenate([fs.T, fs.T], 0))
    ones = np.ones([128, 128], NPBF16)

    perm = _qk_row_perm()
    in_maps = []
    for i in range(NCORES):
        rows = slice(OC * i, OC * (i + 1))
        wqi = Wq[rows][perm]                                             # [512, D]
        wki = Wk[rows][perm]
        wqh = np.ascontiguousarray(
            wqi.reshape(NH, 128, KT, 128).transpose(3, 0, 2, 1)).astype(NPBF16)
        wkh = np.ascontiguousarray(
            wki.reshape(NH, 128, KT, 128).transpose(3, 0, 2, 1)).astype(NPBF16)
        wvh = np.ascontiguousarray(
            Wv[rows].reshape(OC, KT, 128).transpose(2, 1, 0)).astype(NPBF16)
        in_maps.append({
            "xT": xT, "wq": wqh, "wk": wkh, "wv": wvh, "wo": woh,
            "cs": csh, "sn": snh, "ones": ones,
        })
    return in_maps


_NC_CACHE = None


def _get_nc():
    global _NC_CACHE
    if _NC_CACHE is None:
        _NC_CACHE = build_nc()
    return _NC_CACHE


def _run(in_maps, trace=False):
    nc = _get_nc()
    res = bass_utils.run_bass_kernel_spmd(
        nc, in_maps, core_ids=list(range(NCORES)), trace=trace)
    return res


def _assemble(results):
    out = np.empty((B, L, D), np.float32)
    for i in range(NCORES):
        o = results[i]["out"]                       # [D, SH] f32
        for b in range(B):
            out[b, SHB * i:SHB * (i + 1), :] = o[:, b * SHB:(b + 1) * SHB].T
    return out


def kernel(x, freqs_cos, freqs_sin, Wq, Wk, Wv, Wo):
    in_maps = _prep_inputs(x, freqs_cos, freqs_sin, Wq, Wk, Wv, Wo)
    res = _run(in_maps, trace=False)
    return _assemble(res.results)
